# revision 9
# baseline (speedup 1.0000x reference)
"""Trainium2 Bass kernel for nn_CognitiveNetwork (moe_routing).

Strategy: data-parallel over batch across 8 NeuronCores (each core gets
B/8 = 256 rows; the network is batch-row independent so no collectives).
Activations live feature-on-partition ([h, b] layout, 8 chunks of 128
partitions x 256 batch cols).

v3 over v2 (HW-measured matmul rates: fp16 = 1 col/cyc, fp8 DoubleRow =
2 K-rows/cyc i.e. 2x fp16; the local cost model's 4x DR pricing is wrong):
- ih matmuls for the i/f/o gates run fp8e4 DoubleRow (the g gate stays
  fp16: numpy ablation shows e4m3 on g alone costs 3.6e-2 rel err vs
  9.5e-3/3.4e-3/9.9e-3 for i/f/o; all-four would be 3.9e-2 against the
  2e-2 gate, i+f+o keeps the total at ~1.35e-2).
- LayerNorm stats chain shortened with a single ACT Rsqrt (was
  Sqrt + DVE reciprocal); on repeat steps the hh matmuls queue first and
  now fully cover the stats->ln latency.
- First-occurrence steps (no hh to hide latency) defer the rstd multiply
  past the gate matmuls: ih consumes dds = (mu - p) directly (ready one
  DVE op after the stats matmul, ~2us before ln would be), and the
  per-batch-column rstd (replicated across partitions by the ones-matmul
  trick) multiplies the gate psums on the DVE before activation.
- x input arrives fp16 (halves the startup DMA; xi is fp16 anyway) and
  step 0's first matmuls read it directly (no copy).
- The output DMA streams per pair during the last step's association
  instead of one 4MB transfer after everything.
- LSTM state rides DRAM in fp16/fp8 with loads prefetched a step early;
  weight DMAs for step t+1 are issued throughout step t against deeper
  tile pools (each PE stall also costs ~3us of tensor-engine P-state
  ramp-down).
"""

import numpy as np
import ml_dtypes

import concourse.bass as bass
import concourse.mybir as mybir
import concourse.tile as tile
from concourse.bass_utils import run_bass_kernel_spmd

F16NP = np.float16
F8NP = ml_dtypes.float8_e4m3
F32 = mybir.dt.float32
F16 = mybir.dt.float16
F8 = mybir.dt.float8e4
AF = mybir.ActivationFunctionType
OP = mybir.AluOpType
DR = mybir.MatmulPerfMode.DoubleRow

P = 128
H = 1024
NCH = H // P            # 8 feature chunks
NCORES = 8
LN_EPS = 1e-5
GORD = [0, 2, 3, 1]     # packed gate order (i, g, o, f) from torch (i,f,g,o)
F8GATES = [(0, 0), (1, 2), (2, 3)]   # (fp8 tensor gidx, packed gg): i, o, f
S_LN = 8.0              # ln scale (folded into rstd)
S_IH = 8.0              # W_ih scale -> ih psum x64 (fp16 and fp8 paths)
S_HH = 4.0              # W_hh fp8 scale
S_H = 16.0              # h fp8 scale  -> hh psum x64
PS_INV = 1.0 / 64.0

_wsplit_ctr = [0]


def _split_multi_waits(nc):
    """This container's walrus codegen accepts at most ONE sem wait per
    instruction; TileContext attaches several to drains/ops. Split extras
    into preceding same-engine nops (same semantics: engine streams are
    in-order, so waits on a directly preceding nop gate the instruction)."""
    n_split = 0
    for f in nc.m.functions:
        for bb in f.blocks:
            if not any(
                i.sync_info and i.sync_info.on_wait and len(i.sync_info.on_wait) > 1
                for i in bb.instructions
            ):
                continue
            new = []
            for inst in bb.instructions:
                si = inst.sync_info
                waits = list(si.on_wait) if si and si.on_wait else []
                if len(waits) > 1:
                    n_split += 1
                    for w in waits[:-1]:
                        _wsplit_ctr[0] += 1
                        new.append(mybir.InstNoOp(
                            name=f"I-wsplit-{_wsplit_ctr[0]}",
                            engine=inst.engine, ins=[], outs=[],
                            sync_info=mybir.SyncInfo(on_wait=[w], on_update=[]),
                        ))
                    inst.sync_info = mybir.SyncInfo(
                        on_wait=[waits[-1]], on_update=list(si.on_update or []))
                new.append(inst)
            bb.instructions = new
    return n_split


def _analyze(seq):
    """Occurrence structure of the routed cell sequence."""
    slots = []                      # cells in first-use order
    slot_of = {}
    occs = {}                       # cell -> [step indices]
    for t, e in enumerate(seq):
        if e not in slot_of:
            slot_of[e] = len(slots)
            slots.append(e)
        occs.setdefault(e, []).append(t)
    rep_cells = [e for e in slots if len(occs[e]) > 1]
    rep_slot_of = {e: i for i, e in enumerate(rep_cells)}
    plan = []
    for t, e in enumerate(seq):
        o = occs[e]
        k = o.index(t)
        first = (k == 0)
        prev_adj = (not first) and (o[k - 1] == t - 1)
        load = (not first) and not prev_adj
        store = (k + 1 < len(o)) and (o[k + 1] > t + 1)
        pass_sbuf = (k + 1 < len(o)) and (o[k + 1] == t + 1)
        plan.append(dict(cell=e, slot=slot_of[e],
                         rep_slot=rep_slot_of.get(e), first=first,
                         prev_adj=prev_adj, load=load, store=store,
                         pass_sbuf=pass_sbuf))
    return slots, rep_cells, plan


def _pack_feat_cols(v2d):
    """[n, C*P] per-cell row-major -> [P, n*C] (partition = within-chunk idx,
    col = cell*C + chunk)."""
    n, tot = v2d.shape
    C = tot // P
    return np.ascontiguousarray(
        v2d.reshape(n, C, P).transpose(2, 0, 1).reshape(P, n * C).astype(np.float32))


def _host_pack(inputs, slots, rep_cells):
    """Transpose/cast/scale/pack all weights on the host (once per call)."""
    W_p = np.asarray(inputs["W_p"], np.float32)
    W_ih = np.asarray(inputs["W_ih"], np.float32)
    W_hh = np.asarray(inputs["W_hh"], np.float32)
    W_a = np.asarray(inputs["W_a"], np.float32)
    gamma = np.asarray(inputs["gamma"], np.float32)
    beta = np.asarray(inputs["beta"], np.float32)

    def pack_square(W, cells):
        # W[e]: [H(out), H(in)] -> lhsT view [in, out] -> [P, ic, o]
        out = np.empty((len(cells), P, NCH * H), F16NP)
        for i, e in enumerate(cells):
            t = W[e].T.reshape(NCH, P, H).transpose(1, 0, 2)   # [P, ic, o]
            out[i] = t.reshape(P, NCH * H).astype(F16NP)
        return out

    def pack_ihg(cells):
        # fp16 g gate (torch row block [2H:3H)), negated + gamma-folded:
        # the device computes dds = (mu - p) (one op shorter chain) and
        # (-W)(-ln) restores the sign in the psum. Per quarter q:
        # col = ic*256 + hf*128 + c.
        out = np.empty((len(cells), 4, P, 2 * H), F16NP)
        for i, e in enumerate(cells):
            w = W_ih[e][2 * H:3 * H] * gamma[e][None, :] * (-S_IH)
            a = w.reshape(4, 2, P, NCH, P)           # [q, hf, c, ic, p]
            a = a.transpose(0, 4, 3, 1, 2)           # [q, p, ic, hf, c]
            out[i] = a.reshape(4, P, 2 * H).astype(F16NP)
        return out

    def pack_ih8(cells):
        # fp8 DoubleRow lhsT for gates (i, o, f) = torch rows [0, 3, 1]:
        # per (q, gidx, hf, jpair): [K=128, 2, 128].
        # col = gidx*2048 + hf*1024 + j*256 + i*128 + m. First-occurrence
        # steps DMA only cols [0, 4096) (i, o).
        out = np.empty((len(cells), 4, P, 3 * 2048), F8NP)
        for i, e in enumerate(cells):
            w = W_ih[e] * gamma[e][None, :] * (-S_IH)
            b = w.reshape(4, 4, 2, P, 4, 2, P)       # [tg, q, hf, m, j, i, k]
            b = b[[0, 3, 1]]
            b = b.transpose(1, 6, 0, 2, 4, 5, 3)     # [q, k, gidx, hf, j, i, m]
            out[i] = b.reshape(4, P, 3 * 2048).astype(F8NP)
        return out

    def pack_hh(cells):
        # fp8 DoubleRow lhsT: per (q, gg, hf, jpair): [K=128, 2, 128].
        # col = gg*2048 + hf*1024 + j*256 + i*128 + m, scaled by S_HH.
        out = np.empty((len(cells), 4, P, NCH * H), F8NP)
        for i, e in enumerate(cells):
            w = W_hh[e] * S_HH
            b = w.reshape(4, 4, 2, P, 4, 2, P)       # [g, q, hf, m, j, i, k]
            b = b[GORD]
            b = b.transpose(1, 6, 0, 2, 4, 5, 3)     # [q, k, g, hf, j, i, m]
            out[i] = b.reshape(4, P, NCH * H).astype(F8NP)
        return out

    b_ih = np.asarray(inputs["b_ih"], np.float32)
    b_hh = np.asarray(inputs["b_hh"], np.float32)
    # gate bias with beta folded through W_ih; reordered gate-major (GORD)
    bg = np.stack([b_ih[e] + b_hh[e] + W_ih[e] @ beta[e] for e in slots])
    bg = bg.reshape(len(slots), 4, H)[:, GORD].reshape(len(slots), 4 * H)
    bp = np.asarray(inputs["b_p"], np.float32)[slots]
    ba = np.asarray(inputs["b_a"], np.float32)[slots]
    biases_zero = (not bg.any()) and (not bp.any()) and (not ba.any())
    return dict(
        wp=pack_square(W_p, slots),
        wa=pack_square(W_a, slots),
        wihg=pack_ihg(slots),
        wih8=pack_ih8(slots),
        whh=pack_hh(rep_cells) if rep_cells else None,
        bp=_pack_feat_cols(bp),
        bg=_pack_feat_cols(bg),
        ba=_pack_feat_cols(ba),
        biases_zero=biases_zero,
    )


def _build(plan, n_used, n_rep, Bl, gate_sig, n_steps, biases_zero,
           n_emit=None, n_reps=1):
    """Emit the Bass program (shared by all 8 cores; per-core x differs).

    n_reps > 1 wraps the whole step sequence in a hardware For_i loop —
    used for timing (per-iteration wall slope cancels the ~83ms axon
    dispatch overhead). Every iteration recomputes identically from x, so
    the output stays correct."""
    nc = bass.Bass()
    BW = NCH * Bl                                    # 2048 free cols
    B2 = 2 * Bl

    xin_d = nc.dram_tensor("xin", [P, BW], F16, kind="ExternalInput")
    wp_d = nc.dram_tensor("wp", [n_used, P, NCH * H], F16, kind="ExternalInput")
    wa_d = nc.dram_tensor("wa", [n_used, P, NCH * H], F16, kind="ExternalInput")
    wihg_d = nc.dram_tensor("wihg", [n_used, 4, P, 2 * H], F16, kind="ExternalInput")
    wih8_d = nc.dram_tensor("wih8", [n_used, 4, P, 3 * 2048], F8, kind="ExternalInput")
    whh_d = (nc.dram_tensor("whh", [n_rep, 4, P, NCH * H], F8, kind="ExternalInput")
             if n_rep else None)
    bp_d = nc.dram_tensor("bp", [P, n_used * NCH], F32, kind="ExternalInput")
    bg_d = nc.dram_tensor("bg", [P, n_used * 4 * NCH], F32, kind="ExternalInput")
    ba_d = nc.dram_tensor("ba", [P, n_used * NCH], F32, kind="ExternalInput")
    out_d = nc.dram_tensor("out", [P, BW], F32, kind="ExternalOutput")

    if n_emit is None:
        n_emit = n_steps
    import contextlib
    with tile.TileContext(nc) as tc:
        with (
            tc.tile_pool(name="const", bufs=1) as constp,
            tc.tile_pool(name="sb", bufs=2) as sb,
            tc.tile_pool(name="wpool", bufs=2) as wpool,
            tc.tile_pool(name="psum", bufs=7, space="PSUM") as psum,
            tc.tile_pool(name="dram", bufs=1, space="DRAM") as dram,
        ):
            # ---- persistent tiles -------------------------------------
            x_sb = constp.tile([P, BW], F16, name="x_sb")
            bp_sb = constp.tile([P, n_used * NCH], F32, name="bp_sb")
            bg_sb = constp.tile([P, n_used * 4 * NCH], F32, name="bg_sb")
            ba_sb = constp.tile([P, n_used * NCH], F32, name="ba_sb")
            ones128 = constp.tile([P, P], F16, name="ones128")
            nc.vector.memset(ones128[:, :], 1.0)
            eps64_sb = constp.tile([P, 1], F32, name="eps64_sb")
            nc.vector.memset(eps64_sb[:, 0:1], float(LN_EPS / 64.0))
            epsx64_sb = constp.tile([P, 1], F32, name="epsx64_sb")
            nc.vector.memset(epsx64_sb[:, 0:1], float(LN_EPS * 64.0))
            v_sb = constp.tile([P, BW], F32, name="v_sb")

            # DRAM scratch for recurring-cell LSTM state (h: fp8 x16, c: f16)
            hst = {}
            cst = {}
            for rs in range(n_rep):
                hst[rs] = dram.tile([P, BW], F8, name=f"hst{rs}", tag=f"hst{rs}")
                cst[rs] = dram.tile([P, BW], F16, name=f"cst{rs}", tag=f"cst{rs}")

            def bias1(base, s, oc):
                return base[:, s * NCH + oc: s * NCH + oc + 1]

            def gbias(s, gg, hc):
                c0 = s * 4 * NCH + gg * NCH + hc
                return bg_sb[:, c0: c0 + 1]

            # ---- weight prefetch machinery ----------------------------
            # pend[t] holds the SBUF tiles DMA'd ahead for step t.
            pend = {}

            def fetch_step(t):
                """Job closures that allocate + dma_start step t's inputs."""
                if t >= n_emit:
                    return []
                st = plan[t]
                s = st["slot"]
                d = pend.setdefault(t, {})
                jobs = []

                def jwp():
                    w = wpool.tile([P, NCH * H], F16, name=f"wp{t}", tag="smallw",
                                   bufs=2)
                    nc.sync.dma_start(w[:, :], wp_d[s, :, :])
                    d["wp"] = w

                jobs.append(jwp)

                def mk_jq(q):
                    def jq():
                        n8 = 3 * 2048 if not st["first"] else 2 * 2048
                        w8i = wpool.tile([P, 3 * 2048], F8, name=f"wih8_{t}_{q}",
                                         tag="w8ih", bufs=3)
                        nc.sync.dma_start(w8i[:, 0:n8], wih8_d[s, q, :, 0:n8])
                        d.setdefault("wih8", {})[q] = w8i
                        wg = wpool.tile([P, 2 * H], F16, name=f"wihg{t}_{q}",
                                        tag="wg", bufs=3)
                        nc.sync.dma_start(wg[:, :], wihg_d[s, q, :, :])
                        d.setdefault("wihg", {})[q] = wg
                        if not st["first"]:
                            w8 = wpool.tile([P, NCH * H], F8, name=f"whh{t}_{q}",
                                            tag="w8", bufs=3)
                            nc.sync.dma_start(w8[:, :], whh_d[st["rep_slot"], q, :, :])
                            d.setdefault("whh", {})[q] = w8
                    return jq

                for q in range(4):
                    jobs.append(mk_jq(q))
                    if q == 0 and st["load"]:
                        def jh():
                            h8 = sb.tile([P, BW], F8, name=f"hin{t}", tag="h8load")
                            nc.sync.dma_start(h8[:, :], hst[st["rep_slot"]][:, :])
                            d["h8"] = h8
                        jobs.append(jh)
                    if q == 1 and st["load"]:
                        def jc():
                            ct = sb.tile([P, BW], F16, name=f"cin{t}", tag="c")
                            nc.sync.dma_start(ct[:, :], cst[st["rep_slot"]][:, :])
                            d["c"] = ct
                        jobs.append(jc)

                def jwa():
                    w = wpool.tile([P, NCH * H], F16, name=f"wa{t}", tag="smallw",
                                   bufs=2)
                    nc.sync.dma_start(w[:, :], wa_d[s, :, :])
                    d["wa"] = w

                jobs.append(jwa)
                return jobs

            # preamble order: wp(0) and x feed the first matmuls;
            # remaining step-0 weights next; biases last (needed mid-step)
            nc.sync.dma_start(x_sb[:, :], xin_d[:, :])
            if n_reps == 1:
                jobs0 = fetch_step(0)
                jobs0[0]()                          # wp(0)
                for job in jobs0[1:]:
                    job()
            nc.sync.dma_start(bp_sb[:, :], bp_d[:, :])
            nc.sync.dma_start(bg_sb[:, :], bg_d[:, :])
            nc.sync.dma_start(ba_sb[:, :], ba_d[:, :])

            sbuf_state = {}   # cell -> (h16, h8, c) tiles from prev step

            # n_reps>1: rotating-pool tiles written outside a For_i deadlock
            # when read inside, so step 0's fetches move into the loop body
            # (a small per-iteration pipeline bubble, ~1.5% conservative
            # bias on the timing estimate).
            loop_cm = (tc.For_i(0, n_reps, 1) if n_reps > 1
                       else contextlib.nullcontext())
            with loop_cm:
              if n_reps > 1:
                pend.clear()
                for job in fetch_step(0):
                    job()
              xi_t = x_sb                           # step 0: xi = x (ctx=0)
              for t in range(n_emit):
                st = plan[t]
                s = st["slot"]
                first = st["first"]
                use_hh = not first
                d = pend[t]
                nxt = fetch_step(t + 1)   # jobs to interleave through step t
                nj = iter(nxt)

                def kick(n=1):
                    for _ in range(n):
                        j = next(nj, None)
                        if j is not None:
                            j()

                # ---- LSTM state in ------------------------------------
                if st["load"]:
                    h8_t = d["h8"]
                    c_t = d["c"]
                elif st["prev_adj"]:
                    _, h8_t, c_t = sbuf_state[st["cell"]]
                else:
                    h8_t = None
                    c_t = sb.tile([P, BW], F16, name=f"cnew{t}", tag="c")

                wp_t = d["wp"]

                # ---- perception: p = relu(W_p @ xi + b_p), p2 = p*p ----
                # p16 layout: col = oc*512 + {0:p,256:p2} + b
                p16 = sb.tile([P, NCH, 2, Bl], F16, name=f"p16_{t}", tag="p16")
                stat_ps = psum.tile([P, B2], F32, name=f"st{t}", tag="st", bufs=1)
                for pair in range(4):
                    ps = psum.tile([P, B2], F32, name=f"pp{t}_{pair}", tag="mm")
                    split = (pair == 3)   # last pair: per-half so the stats
                    #                       tail starts one relu earlier
                    for hf in range(2):
                        oc = pair * 2 + hf
                        for ic in range(NCH):
                            nc.tensor.matmul(
                                ps[:, hf * Bl:(hf + 1) * Bl],
                                wp_t[:, ic * H + oc * P: ic * H + (oc + 1) * P],
                                xi_t[:, ic * Bl:(ic + 1) * Bl],
                                start=(ic == 0 and (split or hf == 0)),
                                stop=(ic == NCH - 1 and (split or hf == 1)))
                        if split or hf == 1:
                            segs = ([(hf, slice(hf * Bl, (hf + 1) * Bl))]
                                    if split else [(0, slice(0, Bl)),
                                                   (1, slice(Bl, B2))])
                            for sh, ss in segs:
                                oc2 = pair * 2 + sh
                                if biases_zero and not split:
                                    pv3 = p16[:, 2 * pair:2 * pair + 2, :, :]
                                    nc.scalar.activation(pv3[:, :, 0, :],
                                                         ps[:, :], AF.Relu)
                                    nc.vector.tensor_mul(pv3[:, :, 1, :],
                                                         pv3[:, :, 0, :],
                                                         pv3[:, :, 0, :])
                                    for hf2 in range(2):
                                        occ = pair * 2 + hf2
                                        nc.tensor.matmul(
                                            stat_ps[:, :], ones128[:, :],
                                            p16[:, occ, :, :],
                                            start=(occ == 0),
                                            stop=(occ == NCH - 1))
                                    break
                                nc.scalar.activation(
                                    p16[:, oc2, 0, :], ps[:, ss], AF.Relu,
                                    **({} if biases_zero else
                                       dict(bias=bias1(bp_sb, s, oc2))))
                                nc.vector.tensor_mul(p16[:, oc2, 1, :],
                                                     p16[:, oc2, 0, :],
                                                     p16[:, oc2, 0, :])
                                nc.tensor.matmul(
                                    stat_ps[:, :], ones128[:, :],
                                    p16[:, oc2, :, :],
                                    start=(oc2 == 0), stop=(oc2 == NCH - 1))
                    if pair == 0:
                        kick()   # wp(t+1)

                # ---- gate psum bookkeeping ----------------------------
                gates_q = {}          # q -> {packed gg: psum}
                used_gg = [0, 1, 2] if first else [0, 1, 2, 3]

                def gate_ps(q, gg):
                    gp = gates_q.setdefault(q, {})
                    ps = gp.get(gg)
                    if ps is None:
                        ps = psum.tile([P, B2], F32, name=f"g{t}_{q}_{gg}",
                                       tag="mm")
                        gp[gg] = ps
                    return ps

                def emit_hh(q, ggs=(0, 1, 2, 3)):
                    w8 = d["whh"][q]
                    for gg in ggs:
                        ps = gate_ps(q, gg)
                        for hf in range(2):
                            base = gg * 2048 + hf * 1024
                            dst = ps[:, hf * Bl:(hf + 1) * Bl]
                            for j in range(4):
                                lw = w8[:, base + j * 256: base + (j + 1) * 256]
                                rh = h8_t[:, 2 * j * Bl:(2 * j + 2) * Bl]
                                nc.tensor.matmul(
                                    dst,
                                    lw.rearrange("p (two m) -> p two m", two=2),
                                    rh.rearrange("p (two n) -> p two n", two=2),
                                    start=(hf == 0 and j == 0), stop=False,
                                    perf_mode=DR)

                # fp8 ih matmul: one (hf, j) unit for packed gate gg
                def mm8(q, gidx, gg, hf, j, start, stop):
                    ps = gate_ps(q, gg)
                    col = gidx * 2048 + hf * 1024 + j * 256
                    lw = d["wih8"][q][:, col: col + 256]
                    rh = ln8[:, 2 * j * Bl:(2 * j + 2) * Bl]
                    nc.tensor.matmul(
                        ps[:, hf * Bl:(hf + 1) * Bl],
                        lw.rearrange("p (two m) -> p two m", two=2),
                        rh.rearrange("p (two n) -> p two n", two=2),
                        start=start, stop=stop, perf_mode=DR)

                # fp16 g-gate ih matmul: one (ic, hf) unit
                def mmg(q, ic, hf, start, stop):
                    ps = gate_ps(q, 1)
                    col = ic * 256 + hf * 128
                    nc.tensor.matmul(
                        ps[:, hf * Bl:(hf + 1) * Bl],
                        d["wihg"][q][:, col: col + P],
                        ln_t[:, ic * Bl:(ic + 1) * Bl],
                        start=start, stop=stop)

                # ---- hh for q0 queued before the stats chain's consumers
                # so the PE stays busy through it -----------------------
                if use_hh:
                    emit_hh(0)

                # ---- stats chain (all [P,256], psum-replicated) --------
                # raw = H*SS - S^2 = (H*std)^2
                musq2 = sb.tile([P, Bl], F32, name=f"mq{t}", tag="musq")
                nc.scalar.square(musq2[:, :], stat_ps[:, 0:Bl])
                rawv = sb.tile([P, Bl], F32, name=f"vr{t}", tag="vart")
                nc.vector.scalar_tensor_tensor(
                    rawv[:, :], stat_ps[:, Bl:B2], float(H), musq2[:, :],
                    op0=OP.mult, op1=OP.subtract)
                rstd_t = sb.tile([P, Bl], F16, name=f"rs{t}", tag="rstd")
                if use_hh:
                    # rstd_t = 8/std : ln16/ln8 carry 8x ln
                    nc.scalar.activation(rstd_t[:, :], rawv[:, :], AF.Rsqrt,
                                         bias=eps64_sb[:, 0:1],
                                         scale=float(1.0 / (64.0 * H * H)))
                else:
                    # deferred: rstd_t = 1/(8*std) multiplies the gate psums
                    # (which carry 8x(p-mu)@Wgamma.T) after the matmuls
                    nc.scalar.activation(rstd_t[:, :], rawv[:, :], AF.Rsqrt,
                                         bias=epsx64_sb[:, 0:1],
                                         scale=float(64.0 / (H * H)))

                # ---- matmul moving operands ---------------------------
                # repeat: ln16/ln8 = 8*(mu-p)*rstd (sign folded into W)
                # first:  dds16/dd8 = (mu-p); rstd deferred past the psums
                ln_t = sb.tile([P, BW], F16, name=f"ln{t}", tag="ln")
                ln8 = sb.tile([P, BW], F8, name=f"ln8_{t}", tag="ln8")
                for ic in range(NCH):
                    if use_hh:
                        dd = sb.tile([P, Bl], F16, name=f"d{t}_{ic}", tag="lnd",
                                     bufs=3)
                        nc.vector.scalar_tensor_tensor(
                            dd[:, :], stat_ps[:, 0:Bl], 1.0 / H,
                            p16[:, ic, 0, :], op0=OP.mult, op1=OP.subtract)
                        nc.vector.tensor_mul(ln_t[:, ic * Bl:(ic + 1) * Bl],
                                             dd[:, :], rstd_t[:, :])
                        nc.vector.tensor_mul(ln8[:, ic * Bl:(ic + 1) * Bl],
                                             dd[:, :], rstd_t[:, :])
                    else:
                        nc.vector.scalar_tensor_tensor(
                            ln_t[:, ic * Bl:(ic + 1) * Bl], stat_ps[:, 0:Bl],
                            1.0 / H, p16[:, ic, 0, :],
                            op0=OP.mult, op1=OP.subtract)
                        nc.vector.scalar_tensor_tensor(
                            ln8[:, ic * Bl:(ic + 1) * Bl], stat_ps[:, 0:Bl],
                            1.0 / H, p16[:, ic, 0, :],
                            op0=OP.mult, op1=OP.subtract)

                # ---- gates + LSTM pointwise, per quarter ---------------
                hnew = sb.tile([P, BW], F16, name=f"hn{t}", tag="hnew")
                h8new = (sb.tile([P, BW], F8, name=f"hn8_{t}", tag="h8new")
                         if st["store"] or st["pass_sbuf"] else None)
                f8g = F8GATES if use_hh else F8GATES[:2]   # first: i, o

                def gact(dst, gg, func, q):
                    """activation from the gate psum (repeat steps)."""
                    gp = gates_q[q]
                    if biases_zero:
                        nc.scalar.activation(dst[:, :], gp[gg][:, :], func,
                                             scale=PS_INV)
                    else:
                        for hf in range(2):
                            hs = slice(hf * Bl, (hf + 1) * Bl)
                            nc.scalar.activation(dst[:, hs], gp[gg][:, hs],
                                                 func,
                                                 bias=gbias(s, gg, 2 * q + hf),
                                                 scale=PS_INV)

                def gact_first(dst, gg, func, q):
                    """deferred-rstd: DVE psum*rstd then activation."""
                    gp = gates_q[q]
                    gm = sb.tile([P, B2], F16, name=f"gm{t}_{q}_{gg}", tag="gm",
                                 bufs=3)
                    for hf in range(2):
                        hs = slice(hf * Bl, (hf + 1) * Bl)
                        nc.vector.tensor_mul(gm[:, hs], gp[gg][:, hs],
                                             rstd_t[:, :])
                    if biases_zero:
                        nc.scalar.activation(dst[:, :], gm[:, :], func)
                    else:
                        for hf in range(2):
                            hs = slice(hf * Bl, (hf + 1) * Bl)
                            nc.scalar.activation(dst[:, hs], gm[:, hs], func,
                                                 bias=gbias(s, gg, 2 * q + hf))

                for q in range(4):
                    if q == 0:
                        # j-outer: consume ln/dds pairs as the DVE produces
                        # them (fp8 pair j needs chunks 2j, 2j+1)
                        for j in range(4):
                            for gidx, gg in f8g:
                                for hf in range(2):
                                    mm8(0, gidx, gg, hf, j,
                                        start=(not use_hh and hf == 0 and j == 0),
                                        stop=(hf == 1 and j == 3))
                            for ic in (2 * j, 2 * j + 1):
                                for hf in range(2):
                                    mmg(0, ic, hf,
                                        start=(not use_hh and ic == 0 and hf == 0),
                                        stop=(ic == NCH - 1 and hf == 1))
                    else:
                        if use_hh:
                            emit_hh(q)
                        # per-gate order (i, g, f, o): each gate's psum
                        # closes early so its activation (and the c/h
                        # chain) overlaps the remaining gates' matmuls
                        for gidx, gg in f8g[:1]:          # i
                            for hf in range(2):
                                for j in range(4):
                                    mm8(q, gidx, gg, hf, j,
                                        start=(not use_hh and hf == 0 and j == 0),
                                        stop=(hf == 1 and j == 3))
                        for ic in range(NCH):             # g (fp16)
                            for hf in range(2):
                                mmg(q, ic, hf,
                                    start=(not use_hh and ic == 0 and hf == 0),
                                    stop=(ic == NCH - 1 and hf == 1))
                        for gidx, gg in (f8g[2:] + f8g[1:2]):   # f then o
                            for hf in range(2):
                                for j in range(4):
                                    mm8(q, gidx, gg, hf, j,
                                        start=(not use_hh and hf == 0 and j == 0),
                                        stop=(hf == 1 and j == 3))
                    kick(2)
                    # pointwise for chunks hc = 2q, 2q+1. ACT queue order is
                    # (i, g, f, tanh(c), o) so the c chain pipelines behind
                    # the o-gate matmuls and h lands right after them.
                    qs = slice(2 * q * Bl, (2 * q + 2) * Bl)
                    tsi = sb.tile([P, B2], F16, name=f"tsi{t}_{q}", tag="tsi")
                    ttg = sb.tile([P, B2], F16, name=f"ttg{t}_{q}", tag="ttg")
                    tso = sb.tile([P, B2], F16, name=f"tso{t}_{q}", tag="tso")
                    ga = gact if use_hh else gact_first
                    ga(tsi, 0, AF.Sigmoid, q)
                    ga(ttg, 1, AF.Tanh, q)
                    if use_hh:
                        tsf = sb.tile([P, B2], F16, name=f"tsf{t}_{q}", tag="tsf")
                        ga(tsf, 3, AF.Sigmoid, q)
                        nc.vector.tensor_mul(tsi[:, :], tsi[:, :], ttg[:, :])
                        nc.vector.tensor_mul(tsf[:, :], tsf[:, :], c_t[:, qs])
                        nc.vector.tensor_add(c_t[:, qs], tsf[:, :], tsi[:, :])
                    else:
                        nc.vector.tensor_mul(c_t[:, qs], tsi[:, :], ttg[:, :])
                    ttc = sb.tile([P, B2], F16, name=f"ttc{t}_{q}", tag="ttc")
                    nc.scalar.activation(ttc[:, :], c_t[:, qs], AF.Tanh)
                    ga(tso, 2, AF.Sigmoid, q)
                    nc.vector.tensor_mul(hnew[:, qs], tso[:, :], ttc[:, :])
                    if h8new is not None:
                        nc.vector.scalar_tensor_tensor(
                            h8new[:, qs], tso[:, :], S_H, ttc[:, :],
                            op0=OP.mult, op1=OP.mult)

                sbuf_state[st["cell"]] = (hnew, h8new, c_t)

                # ---- LSTM state out -----------------------------------
                if st["store"]:
                    rs = st["rep_slot"]
                    nc.sync.dma_start(hst[rs][:, :], h8new[:, :])
                    nc.sync.dma_start(cst[rs][:, :], c_t[:, :])

                # ---- association: tanh(W_a @ h_new + b_a) --------------
                wa_t = d["wa"]
                # ctx_t = 0.8^t * v_t ; v_t = v_{t-1} + 0.2*g*0.8^{-t}*tanh_t
                ccoef = float(0.2 * gate_sig[s] * (0.8 ** (-t)))
                acoef = float(0.2 * (0.8 ** t))
                if t + 1 < n_emit:
                    xi_t = sb.tile([P, BW], F16, name=f"xi{t + 1}", tag="xi")
                for pair in range(4):
                    ps = psum.tile([P, B2], F32, name=f"pa{t}_{pair}", tag="mm")
                    for hf in range(2):
                        oc = pair * 2 + hf
                        for ic in range(NCH):
                            nc.tensor.matmul(
                                ps[:, hf * Bl:(hf + 1) * Bl],
                                wa_t[:, ic * H + oc * P: ic * H + (oc + 1) * P],
                                hnew[:, ic * Bl:(ic + 1) * Bl],
                                start=(hf == 0 and ic == 0),
                                stop=(hf == 1 and ic == NCH - 1))
                    # fused tail: tanh -> v update -> next xi
                    tnh = sb.tile([P, B2], F16, name=f"tnh{t}_{pair}", tag="tnh")
                    if biases_zero:
                        nc.scalar.activation(tnh[:, :], ps[:, :], AF.Tanh)
                    else:
                        for hf in range(2):
                            oc = pair * 2 + hf
                            nc.scalar.activation(tnh[:, hf * Bl:(hf + 1) * Bl],
                                                 ps[:, hf * Bl:(hf + 1) * Bl],
                                                 AF.Tanh, bias=bias1(ba_sb, s, oc))
                    cs = slice(pair * B2, (pair + 1) * B2)
                    if t == 0:
                        nc.vector.tensor_scalar_mul(v_sb[:, cs], tnh[:, :], ccoef)
                    else:
                        nc.vector.scalar_tensor_tensor(
                            v_sb[:, cs], tnh[:, :], ccoef, v_sb[:, cs],
                            op0=OP.mult, op1=OP.add)
                    if t + 1 < n_emit:
                        nc.vector.scalar_tensor_tensor(
                            xi_t[:, cs], v_sb[:, cs], acoef, x_sb[:, cs],
                            op0=OP.mult, op1=OP.add)
                    else:
                        # last step: stream the output per pair, overlapped
                        # with the remaining association work
                        nc.sync.dma_start(out_d[:, cs], v_sb[:, cs])
                    if pair == 3:
                        kick()    # wa(t+1)

                kick(8)   # flush any remaining prefetch jobs for t+1

    _split_multi_waits(nc)
    return nc


last_results = None   # BassKernelResults of the most recent run (for test.py)
last_nc = None
last_in_maps = None


def kernel(**inputs):
    n_exec = inputs.pop("_n_exec", None)
    n_reps = int(inputs.pop("_n_reps", 1))
    n_steps = int(inputs.pop("_n_steps", 0)) or None
    seq = [int(v) for v in np.asarray(inputs["cell_indices"]).reshape(-1)]
    if n_steps is None:
        n_steps = len(seq)
    seq = seq[:n_steps]

    x = np.asarray(inputs["x"], np.float32)
    B, Hd = x.shape
    assert Hd == H
    Bl = B // NCORES

    slots, rep_cells, plan = _analyze(seq)
    n_used, n_rep = len(slots), len(rep_cells)
    gl = np.asarray(inputs["gate_logit"], np.float64)
    gate_sig = [1.0 / (1.0 + np.exp(-gl[e])) for e in slots]

    packed = _host_pack(inputs, slots, rep_cells)
    nc = _build(plan, n_used, n_rep, Bl, gate_sig, n_steps,
                packed["biases_zero"], n_emit=n_exec, n_reps=n_reps)

    # per-core input maps (weights identical, x sliced)
    xT = np.ascontiguousarray(x.T)                       # [H, B]
    shared = dict(
        wp=packed["wp"], wa=packed["wa"], wihg=packed["wihg"],
        wih8=packed["wih8"],
        bp=packed["bp"], bg=packed["bg"], ba=packed["ba"])
    if n_rep:
        shared["whh"] = packed["whh"]
    in_maps = []
    for c in range(NCORES):
        xc = xT[:, c * Bl:(c + 1) * Bl]                  # [H, Bl]
        xc = np.ascontiguousarray(
            xc.reshape(NCH, P, Bl).transpose(1, 0, 2).reshape(P, NCH * Bl))
        m = dict(shared)
        m["xin"] = xc.astype(np.float16)
        in_maps.append(m)

    res = run_bass_kernel_spmd(nc, in_maps, core_ids=list(range(NCORES)),
                               trace=False)
    global last_results, last_nc, last_in_maps
    last_results = res
    last_nc = nc
    last_in_maps = in_maps

    scale = np.float64(0.8) ** (n_steps - 1)
    outs = []
    for c in range(NCORES):
        v = res.results[c]["out"]                        # [P, NCH*Bl]
        v = v.reshape(P, NCH, Bl).transpose(1, 0, 2).reshape(H, Bl)
        outs.append(v)
    full = np.concatenate(outs, axis=1)                  # [H, B]
    return np.ascontiguousarray((full.T.astype(np.float64) * scale).astype(np.float32))


# revision 25
# speedup vs baseline: 1.1041x; 1.1041x over previous
"""Trainium2 Bass kernel for nn_CognitiveNetwork (moe_routing).

Strategy: data-parallel over batch across 8 NeuronCores (each core gets
B/8 = 256 rows; the network is batch-row independent so no collectives).
Activations live feature-on-partition ([h, b] layout, 8 chunks of 128
partitions x 256 batch cols).

v3 over v2 (HW-measured matmul rates: fp16 = 1 col/cyc, fp8 DoubleRow =
2 K-rows/cyc i.e. 2x fp16; the local cost model's 4x DR pricing is wrong):
- ih matmuls for the i/f/o gates run fp8e4 DoubleRow (the g gate stays
  fp16: numpy ablation shows e4m3 on g alone costs 3.6e-2 rel err vs
  9.5e-3/3.4e-3/9.9e-3 for i/f/o; all-four would be 3.9e-2 against the
  2e-2 gate, i+f+o keeps the total at ~1.35e-2).
- LayerNorm stats chain shortened with a single ACT Rsqrt (was
  Sqrt + DVE reciprocal); on repeat steps the hh matmuls queue first and
  now fully cover the stats->ln latency.
- First-occurrence steps (no hh to hide latency) defer the rstd multiply
  past the gate matmuls: ih consumes dds = (mu - p) directly (ready one
  DVE op after the stats matmul, ~2us before ln would be), and the
  per-batch-column rstd (replicated across partitions by the ones-matmul
  trick) multiplies the gate psums on the DVE before activation.
- x input arrives fp16 (halves the startup DMA; xi is fp16 anyway) and
  step 0's first matmuls read it directly (no copy).
- The output DMA streams per pair during the last step's association
  instead of one 4MB transfer after everything.
- LSTM state rides DRAM in fp16/fp8 with loads prefetched a step early;
  weight DMAs for step t+1 are issued throughout step t against deeper
  tile pools (each PE stall also costs ~3us of tensor-engine P-state
  ramp-down).
"""

import numpy as np
import ml_dtypes

import concourse.bass as bass
import concourse.mybir as mybir
import concourse.tile as tile
from concourse.bass_utils import run_bass_kernel_spmd

F16NP = np.float16
F8NP = ml_dtypes.float8_e4m3
F32 = mybir.dt.float32
F16 = mybir.dt.float16
F8 = mybir.dt.float8e4
AF = mybir.ActivationFunctionType
OP = mybir.AluOpType
DR = mybir.MatmulPerfMode.DoubleRow

P = 128
H = 1024
NCH = H // P            # 8 feature chunks
NCORES = 8
LN_EPS = 1e-5
GORD = [0, 2, 3, 1]     # packed gate order (i, g, o, f) from torch (i,f,g,o)
F8GATES = [(0, 0), (1, 2), (2, 3)]   # (fp8 tensor gidx, packed gg): i, o, f
S_LN = 8.0              # ln scale (folded into rstd)
S_IH = 8.0              # W_ih scale -> ih psum x64 (fp16 and fp8 paths)
S_HH = 4.0              # W_hh fp8 scale
S_H = 16.0              # h fp8 scale  -> hh psum x64
PS_INV = 1.0 / 64.0

_wsplit_ctr = [0]


def _split_multi_waits(nc):
    """This container's walrus codegen accepts at most ONE sem wait per
    instruction; TileContext attaches several to drains/ops. Split extras
    into preceding same-engine nops (same semantics: engine streams are
    in-order, so waits on a directly preceding nop gate the instruction)."""
    n_split = 0
    for f in nc.m.functions:
        for bb in f.blocks:
            if not any(
                i.sync_info and i.sync_info.on_wait and len(i.sync_info.on_wait) > 1
                for i in bb.instructions
            ):
                continue
            new = []
            for inst in bb.instructions:
                si = inst.sync_info
                waits = list(si.on_wait) if si and si.on_wait else []
                if len(waits) > 1:
                    n_split += 1
                    for w in waits[:-1]:
                        _wsplit_ctr[0] += 1
                        new.append(mybir.InstNoOp(
                            name=f"I-wsplit-{_wsplit_ctr[0]}",
                            engine=inst.engine, ins=[], outs=[],
                            sync_info=mybir.SyncInfo(on_wait=[w], on_update=[]),
                        ))
                    inst.sync_info = mybir.SyncInfo(
                        on_wait=[waits[-1]], on_update=list(si.on_update or []))
                new.append(inst)
            bb.instructions = new
    return n_split


def _analyze(seq):
    """Occurrence structure of the routed cell sequence."""
    slots = []                      # cells in first-use order
    slot_of = {}
    occs = {}                       # cell -> [step indices]
    for t, e in enumerate(seq):
        if e not in slot_of:
            slot_of[e] = len(slots)
            slots.append(e)
        occs.setdefault(e, []).append(t)
    rep_cells = [e for e in slots if len(occs[e]) > 1]
    rep_slot_of = {e: i for i, e in enumerate(rep_cells)}
    plan = []
    for t, e in enumerate(seq):
        o = occs[e]
        k = o.index(t)
        first = (k == 0)
        prev_adj = (not first) and (o[k - 1] == t - 1)
        load = (not first) and not prev_adj
        store = (k + 1 < len(o)) and (o[k + 1] > t + 1)
        pass_sbuf = (k + 1 < len(o)) and (o[k + 1] == t + 1)
        plan.append(dict(cell=e, slot=slot_of[e],
                         rep_slot=rep_slot_of.get(e), first=first,
                         prev_adj=prev_adj, load=load, store=store,
                         pass_sbuf=pass_sbuf))
    return slots, rep_cells, plan


def _pack_feat_cols(v2d):
    """[n, C*P] per-cell row-major -> [P, n*C] (partition = within-chunk idx,
    col = cell*C + chunk)."""
    n, tot = v2d.shape
    C = tot // P
    return np.ascontiguousarray(
        v2d.reshape(n, C, P).transpose(2, 0, 1).reshape(P, n * C).astype(np.float32))


def _host_pack(inputs, slots, rep_cells):
    """Transpose/cast/scale/pack all weights on the host (once per call)."""
    W_p = np.asarray(inputs["W_p"], np.float32)
    W_ih = np.asarray(inputs["W_ih"], np.float32)
    W_hh = np.asarray(inputs["W_hh"], np.float32)
    W_a = np.asarray(inputs["W_a"], np.float32)
    gamma = np.asarray(inputs["gamma"], np.float32)
    beta = np.asarray(inputs["beta"], np.float32)

    def pack_square(W, cells):
        # W[e]: [H(out), H(in)] -> lhsT view [in, out] -> [P, ic, o]
        out = np.empty((len(cells), P, NCH * H), F16NP)
        for i, e in enumerate(cells):
            t = W[e].T.reshape(NCH, P, H).transpose(1, 0, 2)   # [P, ic, o]
            out[i] = t.reshape(P, NCH * H).astype(F16NP)
        return out

    def pack_ihg(cells):
        # fp16 g gate (torch row block [2H:3H)), negated + gamma-folded:
        # the device computes dds = (mu - p) (one op shorter chain) and
        # (-W)(-ln) restores the sign in the psum. Per quarter q:
        # col = ic*256 + hf*128 + c.
        out = np.empty((len(cells), 4, P, 2 * H), F16NP)
        for i, e in enumerate(cells):
            w = W_ih[e][2 * H:3 * H] * gamma[e][None, :] * (-S_IH)
            a = w.reshape(4, 2, P, NCH, P)           # [q, hf, c, ic, p]
            a = a.transpose(0, 4, 3, 1, 2)           # [q, p, ic, hf, c]
            out[i] = a.reshape(4, P, 2 * H).astype(F16NP)
        return out

    def pack_ih8(cells):
        # fp8 DoubleRow lhsT for gates (i, o, f) = torch rows [0, 3, 1]:
        # per (q, gidx, hf, jpair): [K=128, 2, 128].
        # col = gidx*2048 + hf*1024 + j*256 + i*128 + m. First-occurrence
        # steps DMA only cols [0, 4096) (i, o).
        out = np.empty((len(cells), 4, P, 3 * 2048), F8NP)
        for i, e in enumerate(cells):
            w = W_ih[e] * gamma[e][None, :] * (-S_IH)
            b = w.reshape(4, 4, 2, P, 4, 2, P)       # [tg, q, hf, m, j, i, k]
            b = b[[0, 3, 1]]
            b = b.transpose(1, 6, 0, 2, 4, 5, 3)     # [q, k, gidx, hf, j, i, m]
            out[i] = b.reshape(4, P, 3 * 2048).astype(F8NP)
        return out

    def pack_hh(cells):
        # fp8 DoubleRow lhsT: per (q, gg, hf, jpair): [K=128, 2, 128].
        # col = gg*2048 + hf*1024 + j*256 + i*128 + m, scaled by S_HH.
        out = np.empty((len(cells), 4, P, NCH * H), F8NP)
        for i, e in enumerate(cells):
            w = W_hh[e] * S_HH
            b = w.reshape(4, 4, 2, P, 4, 2, P)       # [g, q, hf, m, j, i, k]
            b = b[GORD]
            b = b.transpose(1, 6, 0, 2, 4, 5, 3)     # [q, k, g, hf, j, i, m]
            out[i] = b.reshape(4, P, NCH * H).astype(F8NP)
        return out

    b_ih = np.asarray(inputs["b_ih"], np.float32)
    b_hh = np.asarray(inputs["b_hh"], np.float32)
    # gate bias with beta folded through W_ih; reordered gate-major (GORD)
    bg = np.stack([b_ih[e] + b_hh[e] + W_ih[e] @ beta[e] for e in slots])
    bg = bg.reshape(len(slots), 4, H)[:, GORD].reshape(len(slots), 4 * H)
    bp = np.asarray(inputs["b_p"], np.float32)[slots]
    ba = np.asarray(inputs["b_a"], np.float32)[slots]
    biases_zero = (not bg.any()) and (not bp.any()) and (not ba.any())
    return dict(
        wp=pack_square(W_p, slots),
        wa=pack_square(W_a, slots),
        wihg=pack_ihg(slots),
        wih8=pack_ih8(slots),
        whh=pack_hh(rep_cells) if rep_cells else None,
        bp=_pack_feat_cols(bp),
        bg=_pack_feat_cols(bg),
        ba=_pack_feat_cols(ba),
        biases_zero=biases_zero,
    )


def _build(plan, n_used, n_rep, Bl, gate_sig, n_steps, biases_zero,
           n_emit=None, n_reps=1):
    """Emit the Bass program (shared by all 8 cores; per-core x differs).

    n_reps > 1 wraps the whole step sequence in a hardware For_i loop —
    used for timing (per-iteration wall slope cancels the ~83ms axon
    dispatch overhead). Every iteration recomputes identically from x, so
    the output stays correct."""
    nc = bass.Bass()
    BW = NCH * Bl                                    # 2048 free cols
    B2 = 2 * Bl

    xin_d = nc.dram_tensor("xin", [P, BW], F16, kind="ExternalInput")
    wp_d = nc.dram_tensor("wp", [n_used, P, NCH * H], F16, kind="ExternalInput")
    wa_d = nc.dram_tensor("wa", [n_used, P, NCH * H], F16, kind="ExternalInput")
    wihg_d = nc.dram_tensor("wihg", [n_used, 4, P, 2 * H], F16, kind="ExternalInput")
    wih8_d = nc.dram_tensor("wih8", [n_used, 4, P, 3 * 2048], F8, kind="ExternalInput")
    whh_d = (nc.dram_tensor("whh", [n_rep, 4, P, NCH * H], F8, kind="ExternalInput")
             if n_rep else None)
    bp_d = nc.dram_tensor("bp", [P, n_used * NCH], F32, kind="ExternalInput")
    bg_d = nc.dram_tensor("bg", [P, n_used * 4 * NCH], F32, kind="ExternalInput")
    ba_d = nc.dram_tensor("ba", [P, n_used * NCH], F32, kind="ExternalInput")
    out_d = nc.dram_tensor("out", [P, BW], F32, kind="ExternalOutput")

    if n_emit is None:
        n_emit = n_steps
    import contextlib
    with tile.TileContext(nc) as tc:
        with (
            tc.tile_pool(name="const", bufs=1) as constp,
            tc.tile_pool(name="sb", bufs=2) as sb,
            tc.tile_pool(name="wpool", bufs=2) as wpool,
            tc.tile_pool(name="psum", bufs=7, space="PSUM") as psum,
            tc.tile_pool(name="dram", bufs=1, space="DRAM") as dram,
        ):
            # ---- persistent tiles -------------------------------------
            x_sb = constp.tile([P, BW], F16, name="x_sb")
            bp_sb = constp.tile([P, n_used * NCH], F32, name="bp_sb")
            bg_sb = constp.tile([P, n_used * 4 * NCH], F32, name="bg_sb")
            ba_sb = constp.tile([P, n_used * NCH], F32, name="ba_sb")
            ones128 = constp.tile([P, P], F16, name="ones128")
            nc.vector.memset(ones128[:, :], 1.0)
            eps64_sb = constp.tile([P, 1], F32, name="eps64_sb")
            nc.vector.memset(eps64_sb[:, 0:1], float(LN_EPS / 64.0))
            epsx64_sb = constp.tile([P, 1], F32, name="epsx64_sb")
            nc.vector.memset(epsx64_sb[:, 0:1], float(LN_EPS * 64.0))
            v_sb = constp.tile([P, BW], F32, name="v_sb")

            # DRAM scratch for recurring-cell LSTM state (h: fp8 x16, c: f16)
            hst = {}
            cst = {}
            for rs in range(n_rep):
                hst[rs] = dram.tile([P, BW], F8, name=f"hst{rs}", tag=f"hst{rs}")
                cst[rs] = dram.tile([P, BW], F16, name=f"cst{rs}", tag=f"cst{rs}")

            def bias1(base, s, oc):
                return base[:, s * NCH + oc: s * NCH + oc + 1]

            def gbias(s, gg, hc):
                c0 = s * 4 * NCH + gg * NCH + hc
                return bg_sb[:, c0: c0 + 1]

            # ---- weight prefetch machinery ----------------------------
            # pend[t] holds the SBUF tiles DMA'd ahead for step t.
            pend = {}

            def fetch_step(t):
                """Job closures that allocate + dma_start step t's inputs."""
                if t >= n_emit:
                    return []
                st = plan[t]
                s = st["slot"]
                d = pend.setdefault(t, {})
                jobs = []

                def jwp():
                    w = wpool.tile([P, NCH * H], F16, name=f"wp{t}", tag="smallw",
                                   bufs=2)
                    nc.sync.dma_start(w[:, :], wp_d[s, :, :])
                    d["wp"] = w

                jobs.append(jwp)

                def mk_jq(q):
                    def jq():
                        n8 = 3 * 2048 if not st["first"] else 2 * 2048
                        w8i = wpool.tile([P, 3 * 2048], F8, name=f"wih8_{t}_{q}",
                                         tag="w8ih", bufs=3)
                        nc.sync.dma_start(w8i[:, 0:n8], wih8_d[s, q, :, 0:n8])
                        d.setdefault("wih8", {})[q] = w8i
                        wg = wpool.tile([P, 2 * H], F16, name=f"wihg{t}_{q}",
                                        tag="wg", bufs=3)
                        nc.sync.dma_start(wg[:, :], wihg_d[s, q, :, :])
                        d.setdefault("wihg", {})[q] = wg
                        if not st["first"]:
                            w8 = wpool.tile([P, NCH * H], F8, name=f"whh{t}_{q}",
                                            tag="w8", bufs=3)
                            nc.sync.dma_start(w8[:, :], whh_d[st["rep_slot"], q, :, :])
                            d.setdefault("whh", {})[q] = w8
                    return jq

                for q in range(4):
                    jobs.append(mk_jq(q))
                    if q == 0 and st["load"]:
                        def jh():
                            h8 = sb.tile([P, BW], F8, name=f"hin{t}", tag="h8load")
                            nc.sync.dma_start(h8[:, :], hst[st["rep_slot"]][:, :])
                            d["h8"] = h8
                        jobs.append(jh)
                    if q == 1 and st["load"]:
                        def jc():
                            ct = sb.tile([P, BW], F16, name=f"cin{t}", tag="c")
                            nc.sync.dma_start(ct[:, :], cst[st["rep_slot"]][:, :])
                            d["c"] = ct
                        jobs.append(jc)

                def jwa():
                    w = wpool.tile([P, NCH * H], F16, name=f"wa{t}", tag="smallw",
                                   bufs=2)
                    nc.sync.dma_start(w[:, :], wa_d[s, :, :])
                    d["wa"] = w

                jobs.append(jwa)
                return jobs

            # preamble order: wp(0) and x feed the first matmuls;
            # remaining step-0 weights next; biases last (needed mid-step)
            nc.sync.dma_start(x_sb[:, :], xin_d[:, :])
            if n_reps == 1:
                jobs0 = fetch_step(0)
                jobs0[0]()                          # wp(0)
                for job in jobs0[1:]:
                    job()
            nc.sync.dma_start(bp_sb[:, :], bp_d[:, :])
            nc.sync.dma_start(bg_sb[:, :], bg_d[:, :])
            nc.sync.dma_start(ba_sb[:, :], ba_d[:, :])

            sbuf_state = {}   # cell -> (h16, h8, c) tiles from prev step

            # n_reps>1: rotating-pool tiles written outside a For_i deadlock
            # when read inside, so step 0's fetches move into the loop body
            # (a small per-iteration pipeline bubble, ~1.5% conservative
            # bias on the timing estimate).
            loop_cm = (tc.For_i(0, n_reps, 1) if n_reps > 1
                       else contextlib.nullcontext())
            with loop_cm:
              if n_reps > 1:
                pend.clear()
                for job in fetch_step(0):
                    job()
              xi_t = x_sb                           # step 0: xi = x (ctx=0)
              for t in range(n_emit):
                st = plan[t]
                s = st["slot"]
                first = st["first"]
                use_hh = not first
                d = pend[t]
                nxt = fetch_step(t + 1)   # jobs to interleave through step t
                nj = iter(nxt)

                def kick(n=1):
                    for _ in range(n):
                        j = next(nj, None)
                        if j is not None:
                            j()

                # ---- LSTM state in ------------------------------------
                if st["load"]:
                    h8_t = d["h8"]
                    c_t = d["c"]
                elif st["prev_adj"]:
                    _, h8_t, c_t = sbuf_state[st["cell"]]
                else:
                    h8_t = None
                    c_t = sb.tile([P, BW], F16, name=f"cnew{t}", tag="c")

                wp_t = d["wp"]

                # ---- perception: p = relu(W_p @ xi + b_p), p2 = p*p ----
                # p16 layout: col = oc*512 + {0:p,256:p2} + b
                p16 = sb.tile([P, NCH, 2, Bl], F16, name=f"p16_{t}", tag="p16")
                stat_ps = psum.tile([P, B2], F32, name=f"st{t}", tag="st", bufs=1)
                for pair in range(4):
                    ps = psum.tile([P, B2], F32, name=f"pp{t}_{pair}", tag="mm")
                    split = (pair == 3)   # last pair: per-half so the stats
                    #                       tail starts one relu earlier
                    for hf in range(2):
                        oc = pair * 2 + hf
                        for ic in range(NCH):
                            nc.tensor.matmul(
                                ps[:, hf * Bl:(hf + 1) * Bl],
                                wp_t[:, ic * H + oc * P: ic * H + (oc + 1) * P],
                                xi_t[:, ic * Bl:(ic + 1) * Bl],
                                start=(ic == 0 and (split or hf == 0)),
                                stop=(ic == NCH - 1 and (split or hf == 1)))
                        if split or hf == 1:
                            segs = ([(hf, slice(hf * Bl, (hf + 1) * Bl))]
                                    if split else [(0, slice(0, Bl)),
                                                   (1, slice(Bl, B2))])
                            for sh, ss in segs:
                                oc2 = pair * 2 + sh
                                if biases_zero and not split:
                                    pv3 = p16[:, 2 * pair:2 * pair + 2, :, :]
                                    nc.scalar.activation(pv3[:, :, 0, :],
                                                         ps[:, :], AF.Relu)
                                    nc.vector.tensor_mul(pv3[:, :, 1, :],
                                                         pv3[:, :, 0, :],
                                                         pv3[:, :, 0, :])
                                    for hf2 in range(2):
                                        occ = pair * 2 + hf2
                                        nc.tensor.matmul(
                                            stat_ps[:, :], ones128[:, :],
                                            p16[:, occ, :, :],
                                            start=(occ == 0),
                                            stop=(occ == NCH - 1))
                                    break
                                nc.scalar.activation(
                                    p16[:, oc2, 0, :], ps[:, ss], AF.Relu,
                                    **({} if biases_zero else
                                       dict(bias=bias1(bp_sb, s, oc2))))
                                nc.vector.tensor_mul(p16[:, oc2, 1, :],
                                                     p16[:, oc2, 0, :],
                                                     p16[:, oc2, 0, :])
                                # Sum(p) split from Sum(p^2): the last
                                # Sum(p) instruction then waits only on the
                                # relu, starting the ln chain one square
                                # earlier
                                nc.tensor.matmul(
                                    stat_ps[:, 0:Bl], ones128[:, :],
                                    p16[:, oc2, 0, :],
                                    start=(oc2 == 0), stop=False)
                                nc.tensor.matmul(
                                    stat_ps[:, Bl:B2], ones128[:, :],
                                    p16[:, oc2, 1, :],
                                    start=False, stop=(oc2 == NCH - 1))
                    if pair == 0:
                        kick()   # wp(t+1)

                # ---- gate psum bookkeeping ----------------------------
                gates_q = {}          # q -> {packed gg: psum}
                used_gg = [0, 1, 2] if first else [0, 1, 2, 3]

                def gate_ps(q, gg):
                    gp = gates_q.setdefault(q, {})
                    ps = gp.get(gg)
                    if ps is None:
                        ps = psum.tile([P, B2], F32, name=f"g{t}_{q}_{gg}",
                                       tag="mm")
                        gp[gg] = ps
                    return ps

                def emit_hh(q, ggs=(0, 1, 2, 3)):
                    w8 = d["whh"][q]
                    for gg in ggs:
                        ps = gate_ps(q, gg)
                        for hf in range(2):
                            base = gg * 2048 + hf * 1024
                            dst = ps[:, hf * Bl:(hf + 1) * Bl]
                            for j in range(4):
                                lw = w8[:, base + j * 256: base + (j + 1) * 256]
                                rh = h8_t[:, 2 * j * Bl:(2 * j + 2) * Bl]
                                nc.tensor.matmul(
                                    dst,
                                    lw.rearrange("p (two m) -> p two m", two=2),
                                    rh.rearrange("p (two n) -> p two n", two=2),
                                    start=(hf == 0 and j == 0), stop=False,
                                    perf_mode=DR)

                # fp8 ih matmul: one (hf, j) unit for packed gate gg
                def mm8(q, gidx, gg, hf, j, start, stop):
                    ps = gate_ps(q, gg)
                    col = gidx * 2048 + hf * 1024 + j * 256
                    lw = d["wih8"][q][:, col: col + 256]
                    rh = ln8[:, 2 * j * Bl:(2 * j + 2) * Bl]
                    nc.tensor.matmul(
                        ps[:, hf * Bl:(hf + 1) * Bl],
                        lw.rearrange("p (two m) -> p two m", two=2),
                        rh.rearrange("p (two n) -> p two n", two=2),
                        start=start, stop=stop, perf_mode=DR)

                # fp16 g-gate ih matmul: one (ic, hf) unit
                def mmg(q, ic, hf, start, stop):
                    ps = gate_ps(q, 1)
                    col = ic * 256 + hf * 128
                    nc.tensor.matmul(
                        ps[:, hf * Bl:(hf + 1) * Bl],
                        d["wihg"][q][:, col: col + P],
                        ln_t[:, ic * Bl:(ic + 1) * Bl],
                        start=start, stop=stop)

                # ---- hh for q0 (plus q1's i,g) queued before the stats
                # chain's consumers so the PE stays busy through it ------
                if use_hh:
                    emit_hh(0)
                    emit_hh(1, (0, 1))

                # ---- stats chain + matmul moving operands --------------
                # repeat: ln16/ln8 = 8*(mu-p)*rstd (sign folded into W)
                # first:  dds16/dd8 = (mu-p); rstd deferred past the psums
                # The in-order DVE queue is the critical path here: ops
                # that only need stat_ps (dds) must be queued BEFORE ops
                # that wait on the ACT chain (rawv/recip), or they stall.
                ln_t = sb.tile([P, BW], F16, name=f"ln{t}", tag="ln")
                ln8 = sb.tile([P, BW], F8, name=f"ln8_{t}", tag="ln8")
                musq2 = sb.tile([P, Bl], F32, name=f"mq{t}", tag="musq")
                rawv = sb.tile([P, Bl], F32, name=f"vr{t}", tag="vart")
                rstd_t = sb.tile([P, Bl], F16, name=f"rs{t}", tag="rstd")
                stdt = sb.tile([P, Bl], F32, name=f"sd{t}", tag="stdt")

                def emit_rstd_chain():
                    # raw = H*SS - S^2 = (H*std)^2
                    nc.scalar.square(musq2[:, :], stat_ps[:, 0:Bl])
                    nc.vector.scalar_tensor_tensor(
                        rawv[:, :], stat_ps[:, Bl:B2], float(H), musq2[:, :],
                        op0=OP.mult, op1=OP.subtract)
                    if use_hh:
                        # rstd_t = 8/std : ln16/ln8 carry 8x ln
                        nc.scalar.activation(stdt[:, :], rawv[:, :], AF.Sqrt,
                                             bias=eps64_sb[:, 0:1],
                                             scale=float(1.0 / (64.0 * H * H)))
                    else:
                        # deferred: rstd_t = 1/(8*std) scales the gate psums
                        # (which carry 8x(p-mu)@Wgamma.T) after the matmuls
                        nc.scalar.activation(stdt[:, :], rawv[:, :], AF.Sqrt,
                                             bias=epsx64_sb[:, 0:1],
                                             scale=float(64.0 / (H * H)))
                    with nc.allow_low_precision("f16 rstd: 5e-4 rel ok"):
                        nc.vector.reciprocal(rstd_t[:, :], stdt[:, :])

                if use_hh:
                    dds = []
                    for ic in range(2):
                        dd = sb.tile([P, Bl], F16, name=f"d{t}_{ic}", tag="lnd",
                                     bufs=3)
                        nc.vector.scalar_tensor_tensor(
                            dd[:, :], stat_ps[:, 0:Bl], 1.0 / H,
                            p16[:, ic, 0, :], op0=OP.mult, op1=OP.subtract)
                        dds.append(dd)
                    emit_rstd_chain()
                    for ic in range(NCH):
                        if ic >= 2:
                            dd = sb.tile([P, Bl], F16, name=f"d{t}_{ic}",
                                         tag="lnd", bufs=3)
                            nc.vector.scalar_tensor_tensor(
                                dd[:, :], stat_ps[:, 0:Bl], 1.0 / H,
                                p16[:, ic, 0, :], op0=OP.mult, op1=OP.subtract)
                        else:
                            dd = dds[ic]
                        # both on DVE: GPSIMD (which carries the v/xi
                        # updates) is ~2.5x slower per op and ln16 behind
                        # it would stall the q0 g-gate matmuls
                        nc.vector.tensor_mul(ln8[:, ic * Bl:(ic + 1) * Bl],
                                             dd[:, :], rstd_t[:, :])
                        nc.vector.tensor_mul(ln_t[:, ic * Bl:(ic + 1) * Bl],
                                             dd[:, :], rstd_t[:, :])
                else:
                    # dd8/dds16 straight off stat_ps (GPSIMD cannot read
                    # PSUM, so both on DVE); the rstd chain (only needed
                    # by the pointwise) queues after the first few
                    for ic in range(NCH):
                        nc.vector.scalar_tensor_tensor(
                            ln8[:, ic * Bl:(ic + 1) * Bl], stat_ps[:, 0:Bl],
                            1.0 / H, p16[:, ic, 0, :],
                            op0=OP.mult, op1=OP.subtract)
                        nc.vector.scalar_tensor_tensor(
                            ln_t[:, ic * Bl:(ic + 1) * Bl], stat_ps[:, 0:Bl],
                            1.0 / H, p16[:, ic, 0, :],
                            op0=OP.mult, op1=OP.subtract)
                        if ic == 3:
                            emit_rstd_chain()

                # ---- gates + LSTM pointwise, per quarter ---------------
                hnew = sb.tile([P, BW], F16, name=f"hn{t}", tag="hnew")
                h8new = (sb.tile([P, BW], F8, name=f"hn8_{t}", tag="h8new")
                         if st["store"] or st["pass_sbuf"] else None)
                f8g = F8GATES if use_hh else F8GATES[:2]   # first: i, o
                wa_t = d["wa"]
                pa_ps = {}

                def gact(dst, gg, func, q):
                    """activation from the gate psum (repeat steps)."""
                    gp = gates_q[q]
                    if biases_zero:
                        nc.scalar.activation(dst[:, :], gp[gg][:, :], func,
                                             scale=PS_INV)
                    else:
                        for hf in range(2):
                            hs = slice(hf * Bl, (hf + 1) * Bl)
                            nc.scalar.activation(dst[:, hs], gp[gg][:, hs],
                                                 func,
                                                 bias=gbias(s, gg, 2 * q + hf),
                                                 scale=PS_INV)

                def gact_first(dst, gg, func, q):
                    """deferred-rstd: DVE psum*rstd then activation."""
                    gp = gates_q[q]
                    gm = sb.tile([P, B2], F16, name=f"gm{t}_{q}_{gg}", tag="gm",
                                 bufs=3)
                    for hf in range(2):
                        hs = slice(hf * Bl, (hf + 1) * Bl)
                        nc.vector.tensor_mul(gm[:, hs], gp[gg][:, hs],
                                             rstd_t[:, :])
                    if biases_zero:
                        nc.scalar.activation(dst[:, :], gm[:, :], func)
                    else:
                        for hf in range(2):
                            hs = slice(hf * Bl, (hf + 1) * Bl)
                            nc.scalar.activation(dst[:, hs], gm[:, hs], func,
                                                 bias=gbias(s, gg, 2 * q + hf))

                for q in range(4):
                    if q == 0:
                        # j-outer: consume ln/dds pairs as the DVE produces
                        # them (fp8 pair j needs chunks 2j, 2j+1)
                        for j in range(4):
                            for gidx, gg in f8g:
                                for hf in range(2):
                                    mm8(0, gidx, gg, hf, j,
                                        start=(not use_hh and hf == 0 and j == 0),
                                        stop=(hf == 1 and j == 3))
                            for ic in (2 * j, 2 * j + 1):
                                for hf in range(2):
                                    mmg(0, ic, hf,
                                        start=(not use_hh and ic == 0 and hf == 0),
                                        stop=(ic == NCH - 1 and hf == 1))
                    else:
                        if use_hh:
                            emit_hh(q, (2, 3) if q == 1 else (0, 1, 2, 3))
                        # per-gate order (i, g, f, o): each gate's psum
                        # closes early so its activation (and the c/h
                        # chain) overlaps the remaining gates' matmuls
                        for gidx, gg in f8g[:1]:          # i
                            for hf in range(2):
                                for j in range(4):
                                    mm8(q, gidx, gg, hf, j,
                                        start=(not use_hh and hf == 0 and j == 0),
                                        stop=(hf == 1 and j == 3))
                        for ic in range(NCH):             # g (fp16)
                            for hf in range(2):
                                mmg(q, ic, hf,
                                    start=(not use_hh and ic == 0 and hf == 0),
                                    stop=(ic == NCH - 1 and hf == 1))
                        for gidx, gg in (f8g[2:] + f8g[1:2]):   # f then o
                            for hf in range(2):
                                for j in range(4):
                                    mm8(q, gidx, gg, hf, j,
                                        start=(not use_hh and hf == 0 and j == 0),
                                        stop=(hf == 1 and j == 3))
                    kick(2)
                    if q == 3:
                        # association pairs partial (K chunks 0..5, ready
                        # since quarter 2): covers the PE through the q3
                        # pointwise chain so wa doesn't stall on hnew.
                        # First steps get all 4 pairs (longer pointwise
                        # chain; only 3 gate psums live so PSUM fits);
                        # repeat steps 2 (4 gate psums live).
                        for pair in range(2 if use_hh else 4):
                            pa = pa_ps[pair] = psum.tile(
                                [P, B2], F32, name=f"pa{t}_{pair}", tag="mm")
                            for hf in range(2):
                                for ic in range(6):
                                    nc.tensor.matmul(
                                        pa[:, hf * Bl:(hf + 1) * Bl],
                                        wa_t[:, ic * H + (pair * 2 + hf) * P:
                                             ic * H + (pair * 2 + hf + 1) * P],
                                        hnew[:, ic * Bl:(ic + 1) * Bl],
                                        start=(hf == 0 and ic == 0),
                                        stop=False)
                    # pointwise for chunks hc = 2q, 2q+1. ACT queue order is
                    # (i, g, f, tanh(c), o) so the c chain pipelines behind
                    # the o-gate matmuls and h lands right after them.
                    qs = slice(2 * q * Bl, (2 * q + 2) * Bl)
                    tsi = sb.tile([P, B2], F16, name=f"tsi{t}_{q}", tag="tsi")
                    ttg = sb.tile([P, B2], F16, name=f"ttg{t}_{q}", tag="ttg")
                    tso = sb.tile([P, B2], F16, name=f"tso{t}_{q}", tag="tso")
                    ga = gact if use_hh else gact_first
                    ga(tsi, 0, AF.Sigmoid, q)
                    ga(ttg, 1, AF.Tanh, q)
                    if use_hh:
                        tsf = sb.tile([P, B2], F16, name=f"tsf{t}_{q}", tag="tsf")
                        ga(tsf, 3, AF.Sigmoid, q)
                        nc.vector.tensor_mul(tsi[:, :], tsi[:, :], ttg[:, :])
                        nc.vector.tensor_mul(tsf[:, :], tsf[:, :], c_t[:, qs])
                        nc.vector.tensor_add(c_t[:, qs], tsf[:, :], tsi[:, :])
                    else:
                        nc.vector.tensor_mul(c_t[:, qs], tsi[:, :], ttg[:, :])
                    ttc = sb.tile([P, B2], F16, name=f"ttc{t}_{q}", tag="ttc")
                    nc.scalar.activation(ttc[:, :], c_t[:, qs], AF.Tanh)
                    ga(tso, 2, AF.Sigmoid, q)
                    nc.vector.tensor_mul(hnew[:, qs], tso[:, :], ttc[:, :])
                    if h8new is not None:
                        nc.vector.scalar_tensor_tensor(
                            h8new[:, qs], tso[:, :], S_H, ttc[:, :],
                            op0=OP.mult, op1=OP.mult)

                sbuf_state[st["cell"]] = (hnew, h8new, c_t)

                # ---- LSTM state out -----------------------------------
                if st["store"]:
                    rs = st["rep_slot"]
                    nc.sync.dma_start(hst[rs][:, :], h8new[:, :])
                    nc.sync.dma_start(cst[rs][:, :], c_t[:, :])

                # ---- association: tanh(W_a @ h_new + b_a) --------------
                # ctx_t = 0.8^t * v_t ; v_t = v_{t-1} + 0.2*g*0.8^{-t}*tanh_t
                ccoef = float(0.2 * gate_sig[s] * (0.8 ** (-t)))
                acoef = float(0.2 * (0.8 ** t))
                if t + 1 < n_emit:
                    xi_t = sb.tile([P, BW], F16, name=f"xi{t + 1}", tag="xi")
                for pair in range(4):
                    if pair in pa_ps:
                        ps = pa_ps[pair]
                        ics = range(6, NCH)   # finish the partial pair
                    else:
                        ps = psum.tile([P, B2], F32, name=f"pa{t}_{pair}",
                                       tag="mm")
                        ics = range(NCH)
                    for hf in range(2):
                        oc = pair * 2 + hf
                        for ic in ics:
                            nc.tensor.matmul(
                                ps[:, hf * Bl:(hf + 1) * Bl],
                                wa_t[:, ic * H + oc * P: ic * H + (oc + 1) * P],
                                hnew[:, ic * Bl:(ic + 1) * Bl],
                                start=(pair not in pa_ps
                                       and hf == 0 and ic == 0),
                                stop=(hf == 1 and ic == NCH - 1))
                    # fused tail: tanh -> v update -> next xi
                    tnh = sb.tile([P, B2], F16, name=f"tnh{t}_{pair}", tag="tnh")
                    if biases_zero:
                        nc.scalar.activation(tnh[:, :], ps[:, :], AF.Tanh)
                    else:
                        for hf in range(2):
                            oc = pair * 2 + hf
                            nc.scalar.activation(tnh[:, hf * Bl:(hf + 1) * Bl],
                                                 ps[:, hf * Bl:(hf + 1) * Bl],
                                                 AF.Tanh, bias=bias1(ba_sb, s, oc))
                    cs = slice(pair * B2, (pair + 1) * B2)
                    if t == 0:
                        nc.vector.tensor_scalar_mul(v_sb[:, cs], tnh[:, :], ccoef)
                    else:
                        nc.vector.scalar_tensor_tensor(
                            v_sb[:, cs], tnh[:, :], ccoef, v_sb[:, cs],
                            op0=OP.mult, op1=OP.add)
                    if t + 1 < n_emit:
                        nc.vector.scalar_tensor_tensor(
                            xi_t[:, cs], v_sb[:, cs], acoef, x_sb[:, cs],
                            op0=OP.mult, op1=OP.add)
                    else:
                        # last step: stream the output per pair, overlapped
                        # with the remaining association work
                        nc.sync.dma_start(out_d[:, cs], v_sb[:, cs])
                    if pair == 3:
                        kick()    # wa(t+1)

                kick(8)   # flush any remaining prefetch jobs for t+1

    _split_multi_waits(nc)
    return nc


last_results = None   # BassKernelResults of the most recent run (for test.py)
last_nc = None
last_in_maps = None


def kernel(**inputs):
    n_exec = inputs.pop("_n_exec", None)
    n_reps = int(inputs.pop("_n_reps", 1))
    n_steps = int(inputs.pop("_n_steps", 0)) or None
    seq = [int(v) for v in np.asarray(inputs["cell_indices"]).reshape(-1)]
    if n_steps is None:
        n_steps = len(seq)
    seq = seq[:n_steps]

    x = np.asarray(inputs["x"], np.float32)
    B, Hd = x.shape
    assert Hd == H
    Bl = B // NCORES

    slots, rep_cells, plan = _analyze(seq)
    n_used, n_rep = len(slots), len(rep_cells)
    gl = np.asarray(inputs["gate_logit"], np.float64)
    gate_sig = [1.0 / (1.0 + np.exp(-gl[e])) for e in slots]

    packed = _host_pack(inputs, slots, rep_cells)
    nc = _build(plan, n_used, n_rep, Bl, gate_sig, n_steps,
                packed["biases_zero"], n_emit=n_exec, n_reps=n_reps)

    # per-core input maps (weights identical, x sliced)
    xT = np.ascontiguousarray(x.T)                       # [H, B]
    shared = dict(
        wp=packed["wp"], wa=packed["wa"], wihg=packed["wihg"],
        wih8=packed["wih8"],
        bp=packed["bp"], bg=packed["bg"], ba=packed["ba"])
    if n_rep:
        shared["whh"] = packed["whh"]
    in_maps = []
    for c in range(NCORES):
        xc = xT[:, c * Bl:(c + 1) * Bl]                  # [H, Bl]
        xc = np.ascontiguousarray(
            xc.reshape(NCH, P, Bl).transpose(1, 0, 2).reshape(P, NCH * Bl))
        m = dict(shared)
        m["xin"] = xc.astype(np.float16)
        in_maps.append(m)

    res = run_bass_kernel_spmd(nc, in_maps, core_ids=list(range(NCORES)),
                               trace=False)
    global last_results, last_nc, last_in_maps
    last_results = res
    last_nc = nc
    last_in_maps = in_maps

    scale = np.float64(0.8) ** (n_steps - 1)
    outs = []
    for c in range(NCORES):
        v = res.results[c]["out"]                        # [P, NCH*Bl]
        v = v.reshape(P, NCH, Bl).transpose(1, 0, 2).reshape(H, Bl)
        outs.append(v)
    full = np.concatenate(outs, axis=1)                  # [H, B]
    return np.ascontiguousarray((full.T.astype(np.float64) * scale).astype(np.float32))


# revision 26
# speedup vs baseline: 1.2568x; 1.1383x over previous
"""Trainium2 Bass kernel for nn_CognitiveNetwork (moe_routing).

Strategy: data-parallel over batch across 8 NeuronCores (each core gets
B/8 = 256 rows; the network is batch-row independent so no collectives).
Activations live feature-on-partition ([h, b] layout, 8 chunks of 128
partitions x 256 batch cols).

v3 over v2 (HW-measured matmul rates: fp16 = 1 col/cyc, fp8 DoubleRow =
2 K-rows/cyc i.e. 2x fp16; the local cost model's 4x DR pricing is wrong):
- ih matmuls for the i/f/o gates run fp8e4 DoubleRow (the g gate stays
  fp16: numpy ablation shows e4m3 on g alone costs 3.6e-2 rel err vs
  9.5e-3/3.4e-3/9.9e-3 for i/f/o; all-four would be 3.9e-2 against the
  2e-2 gate, i+f+o keeps the total at ~1.35e-2).
- LayerNorm stats chain shortened with a single ACT Rsqrt (was
  Sqrt + DVE reciprocal); on repeat steps the hh matmuls queue first and
  now fully cover the stats->ln latency.
- First-occurrence steps (no hh to hide latency) defer the rstd multiply
  past the gate matmuls: ih consumes dds = (mu - p) directly (ready one
  DVE op after the stats matmul, ~2us before ln would be), and the
  per-batch-column rstd (replicated across partitions by the ones-matmul
  trick) multiplies the gate psums on the DVE before activation.
- x input arrives fp16 (halves the startup DMA; xi is fp16 anyway) and
  step 0's first matmuls read it directly (no copy).
- The output DMA streams per pair during the last step's association
  instead of one 4MB transfer after everything.
- LSTM state rides DRAM in fp16/fp8 with loads prefetched a step early;
  weight DMAs for step t+1 are issued throughout step t against deeper
  tile pools (each PE stall also costs ~3us of tensor-engine P-state
  ramp-down).
"""

import numpy as np
import ml_dtypes

import concourse.bass as bass
import concourse.mybir as mybir
import concourse.tile as tile
from concourse.bass_utils import run_bass_kernel_spmd

F16NP = np.float16
F8NP = ml_dtypes.float8_e4m3
F32 = mybir.dt.float32
F16 = mybir.dt.float16
F8 = mybir.dt.float8e4
AF = mybir.ActivationFunctionType
OP = mybir.AluOpType
DR = mybir.MatmulPerfMode.DoubleRow

P = 128
H = 1024
NCH = H // P            # 8 feature chunks
NCORES = 8
LN_EPS = 1e-5
GORD = [0, 2, 3, 1]     # packed gate order (i, g, o, f) from torch (i,f,g,o)
F8GATES = [(0, 0), (1, 2), (2, 3)]   # (fp8 tensor gidx, packed gg): i, o, f
S_LN = 8.0              # ln scale (folded into rstd)
S_IH = 8.0              # W_ih scale -> ih psum x64 (fp16 and fp8 paths)
S_HH = 4.0              # W_hh fp8 scale
S_H = 16.0              # h fp8 scale  -> hh psum x64
PS_INV = 1.0 / 64.0

_wsplit_ctr = [0]


def _split_multi_waits(nc):
    """This container's walrus codegen accepts at most ONE sem wait per
    instruction; TileContext attaches several to drains/ops. Split extras
    into preceding same-engine nops (same semantics: engine streams are
    in-order, so waits on a directly preceding nop gate the instruction)."""
    n_split = 0
    for f in nc.m.functions:
        for bb in f.blocks:
            if not any(
                i.sync_info and i.sync_info.on_wait and len(i.sync_info.on_wait) > 1
                for i in bb.instructions
            ):
                continue
            new = []
            for inst in bb.instructions:
                si = inst.sync_info
                waits = list(si.on_wait) if si and si.on_wait else []
                if len(waits) > 1:
                    n_split += 1
                    for w in waits[:-1]:
                        _wsplit_ctr[0] += 1
                        new.append(mybir.InstNoOp(
                            name=f"I-wsplit-{_wsplit_ctr[0]}",
                            engine=inst.engine, ins=[], outs=[],
                            sync_info=mybir.SyncInfo(on_wait=[w], on_update=[]),
                        ))
                    inst.sync_info = mybir.SyncInfo(
                        on_wait=[waits[-1]], on_update=list(si.on_update or []))
                new.append(inst)
            bb.instructions = new
    return n_split


def _analyze(seq):
    """Occurrence structure of the routed cell sequence."""
    slots = []                      # cells in first-use order
    slot_of = {}
    occs = {}                       # cell -> [step indices]
    for t, e in enumerate(seq):
        if e not in slot_of:
            slot_of[e] = len(slots)
            slots.append(e)
        occs.setdefault(e, []).append(t)
    rep_cells = [e for e in slots if len(occs[e]) > 1]
    rep_slot_of = {e: i for i, e in enumerate(rep_cells)}
    plan = []
    for t, e in enumerate(seq):
        o = occs[e]
        k = o.index(t)
        first = (k == 0)
        prev_adj = (not first) and (o[k - 1] == t - 1)
        load = (not first) and not prev_adj
        store = (k + 1 < len(o)) and (o[k + 1] > t + 1)
        pass_sbuf = (k + 1 < len(o)) and (o[k + 1] == t + 1)
        plan.append(dict(cell=e, slot=slot_of[e],
                         rep_slot=rep_slot_of.get(e), first=first,
                         prev_adj=prev_adj, load=load, store=store,
                         pass_sbuf=pass_sbuf))
    return slots, rep_cells, plan


def _pack_feat_cols(v2d):
    """[n, C*P] per-cell row-major -> [P, n*C] (partition = within-chunk idx,
    col = cell*C + chunk)."""
    n, tot = v2d.shape
    C = tot // P
    return np.ascontiguousarray(
        v2d.reshape(n, C, P).transpose(2, 0, 1).reshape(P, n * C).astype(np.float32))


def _host_pack(inputs, slots, rep_cells):
    """Transpose/cast/scale/pack all weights on the host (once per call)."""
    W_p = np.asarray(inputs["W_p"], np.float32)
    W_ih = np.asarray(inputs["W_ih"], np.float32)
    W_hh = np.asarray(inputs["W_hh"], np.float32)
    W_a = np.asarray(inputs["W_a"], np.float32)
    gamma = np.asarray(inputs["gamma"], np.float32)
    beta = np.asarray(inputs["beta"], np.float32)

    def pack_square(W, cells):
        # W[e]: [H(out), H(in)] -> lhsT view [in, out] -> [P, ic, o]
        out = np.empty((len(cells), P, NCH * H), F16NP)
        for i, e in enumerate(cells):
            t = W[e].T.reshape(NCH, P, H).transpose(1, 0, 2)   # [P, ic, o]
            out[i] = t.reshape(P, NCH * H).astype(F16NP)
        return out

    def pack_ihg(cells):
        # fp16 g gate (torch row block [2H:3H)), negated + gamma-folded:
        # the device computes dds = (mu - p) (one op shorter chain) and
        # (-W)(-ln) restores the sign in the psum. Per quarter q:
        # col = ic*256 + hf*128 + c.
        out = np.empty((len(cells), 4, P, 2 * H), F16NP)
        for i, e in enumerate(cells):
            w = W_ih[e][2 * H:3 * H] * gamma[e][None, :] * (-S_IH)
            a = w.reshape(4, 2, P, NCH, P)           # [q, hf, c, ic, p]
            a = a.transpose(0, 4, 3, 1, 2)           # [q, p, ic, hf, c]
            out[i] = a.reshape(4, P, 2 * H).astype(F16NP)
        return out

    def pack_ih8(cells):
        # fp8 DoubleRow lhsT for gates (i, o, f) = torch rows [0, 3, 1]:
        # per (q, gidx, hf, jpair): [K=128, 2, 128].
        # col = gidx*2048 + hf*1024 + j*256 + i*128 + m. First-occurrence
        # steps DMA only cols [0, 4096) (i, o).
        out = np.empty((len(cells), 4, P, 3 * 2048), F8NP)
        for i, e in enumerate(cells):
            w = W_ih[e] * gamma[e][None, :] * (-S_IH)
            b = w.reshape(4, 4, 2, P, 4, 2, P)       # [tg, q, hf, m, j, i, k]
            b = b[[0, 3, 1]]
            b = b.transpose(1, 6, 0, 2, 4, 5, 3)     # [q, k, gidx, hf, j, i, m]
            out[i] = b.reshape(4, P, 3 * 2048).astype(F8NP)
        return out

    def pack_hh(cells):
        # fp8 DoubleRow lhsT: per (q, gg, hf, jpair): [K=128, 2, 128].
        # col = gg*2048 + hf*1024 + j*256 + i*128 + m, scaled by S_HH.
        out = np.empty((len(cells), 4, P, NCH * H), F8NP)
        for i, e in enumerate(cells):
            w = W_hh[e] * S_HH
            b = w.reshape(4, 4, 2, P, 4, 2, P)       # [g, q, hf, m, j, i, k]
            b = b[GORD]
            b = b.transpose(1, 6, 0, 2, 4, 5, 3)     # [q, k, g, hf, j, i, m]
            out[i] = b.reshape(4, P, NCH * H).astype(F8NP)
        return out

    b_ih = np.asarray(inputs["b_ih"], np.float32)
    b_hh = np.asarray(inputs["b_hh"], np.float32)
    # gate bias with beta folded through W_ih; reordered gate-major (GORD)
    bg = np.stack([b_ih[e] + b_hh[e] + W_ih[e] @ beta[e] for e in slots])
    bg = bg.reshape(len(slots), 4, H)[:, GORD].reshape(len(slots), 4 * H)
    bp = np.asarray(inputs["b_p"], np.float32)[slots]
    ba = np.asarray(inputs["b_a"], np.float32)[slots]
    biases_zero = (not bg.any()) and (not bp.any()) and (not ba.any())
    return dict(
        wp=pack_square(W_p, slots),
        wa=pack_square(W_a, slots),
        wihg=pack_ihg(slots),
        wih8=pack_ih8(slots),
        whh=pack_hh(rep_cells) if rep_cells else None,
        bp=_pack_feat_cols(bp),
        bg=_pack_feat_cols(bg),
        ba=_pack_feat_cols(ba),
        biases_zero=biases_zero,
    )


def _build(plan, n_used, n_rep, Bl, gate_sig, n_steps, biases_zero,
           n_emit=None, n_reps=1):
    """Emit the Bass program (shared by all 8 cores; per-core x differs).

    n_reps > 1 wraps the whole step sequence in a hardware For_i loop —
    used for timing (per-iteration wall slope cancels the ~83ms axon
    dispatch overhead). Every iteration recomputes identically from x, so
    the output stays correct."""
    nc = bass.Bass()
    BW = NCH * Bl                                    # 2048 free cols
    B2 = 2 * Bl

    xin_d = nc.dram_tensor("xin", [P, BW], F16, kind="ExternalInput")
    wp_d = nc.dram_tensor("wp", [n_used, P, NCH * H], F16, kind="ExternalInput")
    wa_d = nc.dram_tensor("wa", [n_used, P, NCH * H], F16, kind="ExternalInput")
    wihg_d = nc.dram_tensor("wihg", [n_used, 4, P, 2 * H], F16, kind="ExternalInput")
    wih8_d = nc.dram_tensor("wih8", [n_used, 4, P, 3 * 2048], F8, kind="ExternalInput")
    whh_d = (nc.dram_tensor("whh", [n_rep, 4, P, NCH * H], F8, kind="ExternalInput")
             if n_rep else None)
    bp_d = nc.dram_tensor("bp", [P, n_used * NCH], F32, kind="ExternalInput")
    bg_d = nc.dram_tensor("bg", [P, n_used * 4 * NCH], F32, kind="ExternalInput")
    ba_d = nc.dram_tensor("ba", [P, n_used * NCH], F32, kind="ExternalInput")
    out_d = nc.dram_tensor("out", [P, BW], F32, kind="ExternalOutput")

    if n_emit is None:
        n_emit = n_steps
    import contextlib
    with tile.TileContext(nc) as tc:
        with (
            tc.tile_pool(name="const", bufs=1) as constp,
            tc.tile_pool(name="sb", bufs=2) as sb,
            tc.tile_pool(name="wpool", bufs=2) as wpool,
            tc.tile_pool(name="psum", bufs=7, space="PSUM") as psum,
            tc.tile_pool(name="dram", bufs=1, space="DRAM") as dram,
        ):
            # ---- persistent tiles -------------------------------------
            x_sb = constp.tile([P, BW], F16, name="x_sb")
            bp_sb = constp.tile([P, n_used * NCH], F32, name="bp_sb")
            bg_sb = constp.tile([P, n_used * 4 * NCH], F32, name="bg_sb")
            ba_sb = constp.tile([P, n_used * NCH], F32, name="ba_sb")
            ones128 = constp.tile([P, P], F16, name="ones128")
            nc.vector.memset(ones128[:, :], 1.0)
            eps64_sb = constp.tile([P, 1], F32, name="eps64_sb")
            nc.vector.memset(eps64_sb[:, 0:1], float(LN_EPS / 64.0))
            epsx64_sb = constp.tile([P, 1], F32, name="epsx64_sb")
            nc.vector.memset(epsx64_sb[:, 0:1], float(LN_EPS * 64.0))
            v_sb = constp.tile([P, BW], F32, name="v_sb")

            # DRAM scratch for recurring-cell LSTM state (h: fp8 x16, c: f16)
            hst = {}
            cst = {}
            for rs in range(n_rep):
                hst[rs] = dram.tile([P, BW], F8, name=f"hst{rs}", tag=f"hst{rs}")
                cst[rs] = dram.tile([P, BW], F16, name=f"cst{rs}", tag=f"cst{rs}")

            def bias1(base, s, oc):
                return base[:, s * NCH + oc: s * NCH + oc + 1]

            def gbias(s, gg, hc):
                c0 = s * 4 * NCH + gg * NCH + hc
                return bg_sb[:, c0: c0 + 1]

            # ---- weight prefetch machinery ----------------------------
            # pend[t] holds the SBUF tiles DMA'd ahead for step t.
            pend = {}

            def fetch_step(t):
                """Job closures that allocate + dma_start step t's inputs."""
                if t >= n_emit:
                    return []
                st = plan[t]
                s = st["slot"]
                d = pend.setdefault(t, {})
                jobs = []

                def jwp():
                    w = wpool.tile([P, NCH * H], F16, name=f"wp{t}", tag="smallw",
                                   bufs=2)
                    nc.sync.dma_start(w[:, :], wp_d[s, :, :])
                    d["wp"] = w

                jobs.append(jwp)

                def mk_jq(q):
                    def jq():
                        n8 = 3 * 2048 if not st["first"] else 2 * 2048
                        w8i = wpool.tile([P, 3 * 2048], F8, name=f"wih8_{t}_{q}",
                                         tag="w8ih", bufs=3)
                        nc.sync.dma_start(w8i[:, 0:n8], wih8_d[s, q, :, 0:n8])
                        d.setdefault("wih8", {})[q] = w8i
                        wg = wpool.tile([P, 2 * H], F16, name=f"wihg{t}_{q}",
                                        tag="wg", bufs=3)
                        nc.sync.dma_start(wg[:, :], wihg_d[s, q, :, :])
                        d.setdefault("wihg", {})[q] = wg
                        if not st["first"]:
                            w8 = wpool.tile([P, NCH * H], F8, name=f"whh{t}_{q}",
                                            tag="w8", bufs=3)
                            nc.sync.dma_start(w8[:, :], whh_d[st["rep_slot"], q, :, :])
                            d.setdefault("whh", {})[q] = w8
                    return jq

                for q in range(4):
                    jobs.append(mk_jq(q))
                    if q == 0 and st["load"]:
                        def jh():
                            h8 = sb.tile([P, BW], F8, name=f"hin{t}", tag="h8load")
                            nc.sync.dma_start(h8[:, :], hst[st["rep_slot"]][:, :])
                            d["h8"] = h8
                        jobs.append(jh)
                    if q == 1 and st["load"]:
                        def jc():
                            ct = sb.tile([P, BW], F16, name=f"cin{t}", tag="c")
                            nc.sync.dma_start(ct[:, :], cst[st["rep_slot"]][:, :])
                            d["c"] = ct
                        jobs.append(jc)

                def jwa():
                    w = wpool.tile([P, NCH * H], F16, name=f"wa{t}", tag="smallw",
                                   bufs=2)
                    nc.sync.dma_start(w[:, :], wa_d[s, :, :])
                    d["wa"] = w

                jobs.append(jwa)
                return jobs

            # preamble order: wp(0) and x feed the first matmuls;
            # remaining step-0 weights next; biases last (needed mid-step)
            nc.sync.dma_start(x_sb[:, :], xin_d[:, :])
            if n_reps == 1:
                jobs0 = fetch_step(0)
                jobs0[0]()                          # wp(0)
                for job in jobs0[1:]:
                    job()
            nc.sync.dma_start(bp_sb[:, :], bp_d[:, :])
            nc.sync.dma_start(bg_sb[:, :], bg_d[:, :])
            nc.sync.dma_start(ba_sb[:, :], ba_d[:, :])

            sbuf_state = {}   # cell -> (h16, h8, c) tiles from prev step

            # n_reps>1: rotating-pool tiles written outside a For_i deadlock
            # when read inside, so step 0's fetches move into the loop body
            # (a small per-iteration pipeline bubble, ~1.5% conservative
            # bias on the timing estimate).
            loop_cm = (tc.For_i(0, n_reps, 1) if n_reps > 1
                       else contextlib.nullcontext())
            with loop_cm:
              if n_reps > 1:
                pend.clear()
                for job in fetch_step(0):
                    job()
              xi_t = x_sb                           # step 0: xi = x (ctx=0)
              for t in range(n_emit):
                st = plan[t]
                s = st["slot"]
                first = st["first"]
                use_hh = not first
                d = pend[t]
                nxt = fetch_step(t + 1)   # jobs to interleave through step t
                nj = iter(nxt)

                def kick(n=1):
                    for _ in range(n):
                        j = next(nj, None)
                        if j is not None:
                            j()

                # ---- LSTM state in ------------------------------------
                if st["load"]:
                    h8_t = d["h8"]
                    c_t = d["c"]
                elif st["prev_adj"]:
                    _, h8_t, c_t = sbuf_state[st["cell"]]
                else:
                    h8_t = None
                    c_t = sb.tile([P, BW], F16, name=f"cnew{t}", tag="c")

                wp_t = d["wp"]

                # ---- perception: p = relu(W_p @ xi + b_p), p2 = p*p ----
                # p16 layout: col = oc*512 + {0:p,256:p2} + b
                p16 = sb.tile([P, NCH, 2, Bl], F16, name=f"p16_{t}", tag="p16")
                stat_ps = psum.tile([P, B2], F32, name=f"st{t}", tag="st", bufs=1)
                for pair in range(4):
                    ps = psum.tile([P, B2], F32, name=f"pp{t}_{pair}", tag="mm")
                    split = (pair == 3)   # last pair: per-half so the stats
                    #                       tail starts one relu earlier
                    for hf in range(2):
                        oc = pair * 2 + hf
                        for ic in range(NCH):
                            nc.tensor.matmul(
                                ps[:, hf * Bl:(hf + 1) * Bl],
                                wp_t[:, ic * H + oc * P: ic * H + (oc + 1) * P],
                                xi_t[:, ic * Bl:(ic + 1) * Bl],
                                start=(ic == 0 and (split or hf == 0)),
                                stop=(ic == NCH - 1 and (split or hf == 1)))
                        if split or hf == 1:
                            segs = ([(hf, slice(hf * Bl, (hf + 1) * Bl))]
                                    if split else [(0, slice(0, Bl)),
                                                   (1, slice(Bl, B2))])
                            for sh, ss in segs:
                                oc2 = pair * 2 + sh
                                if biases_zero and not split:
                                    pv3 = p16[:, 2 * pair:2 * pair + 2, :, :]
                                    nc.scalar.activation(pv3[:, :, 0, :],
                                                         ps[:, :], AF.Relu)
                                    nc.vector.tensor_mul(pv3[:, :, 1, :],
                                                         pv3[:, :, 0, :],
                                                         pv3[:, :, 0, :])
                                    for hf2 in range(2):
                                        occ = pair * 2 + hf2
                                        nc.tensor.matmul(
                                            stat_ps[:, :], ones128[:, :],
                                            p16[:, occ, :, :],
                                            start=(occ == 0),
                                            stop=(occ == NCH - 1))
                                    break
                                nc.scalar.activation(
                                    p16[:, oc2, 0, :], ps[:, ss], AF.Relu,
                                    **({} if biases_zero else
                                       dict(bias=bias1(bp_sb, s, oc2))))
                                nc.vector.tensor_mul(p16[:, oc2, 1, :],
                                                     p16[:, oc2, 0, :],
                                                     p16[:, oc2, 0, :])
                                # Sum(p) split from Sum(p^2): the last
                                # Sum(p) instruction then waits only on the
                                # relu, starting the ln chain one square
                                # earlier
                                nc.tensor.matmul(
                                    stat_ps[:, 0:Bl], ones128[:, :],
                                    p16[:, oc2, 0, :],
                                    start=(oc2 == 0), stop=False)
                                nc.tensor.matmul(
                                    stat_ps[:, Bl:B2], ones128[:, :],
                                    p16[:, oc2, 1, :],
                                    start=False, stop=(oc2 == NCH - 1))
                    if pair == 0:
                        kick()   # wp(t+1)

                # ---- gate psum bookkeeping ----------------------------
                gates_q = {}          # q -> {packed gg: psum}
                used_gg = [0, 1, 2] if first else [0, 1, 2, 3]

                def gate_ps(q, gg):
                    gp = gates_q.setdefault(q, {})
                    ps = gp.get(gg)
                    if ps is None:
                        ps = psum.tile([P, B2], F32, name=f"g{t}_{q}_{gg}",
                                       tag="mm")
                        gp[gg] = ps
                    return ps

                def emit_hh(q, ggs=(0, 1, 2, 3)):
                    w8 = d["whh"][q]
                    for gg in ggs:
                        ps = gate_ps(q, gg)
                        for hf in range(2):
                            base = gg * 2048 + hf * 1024
                            dst = ps[:, hf * Bl:(hf + 1) * Bl]
                            for j in range(4):
                                lw = w8[:, base + j * 256: base + (j + 1) * 256]
                                rh = h8_t[:, 2 * j * Bl:(2 * j + 2) * Bl]
                                nc.tensor.matmul(
                                    dst,
                                    lw.rearrange("p (two m) -> p two m", two=2),
                                    rh.rearrange("p (two n) -> p two n", two=2),
                                    start=(hf == 0 and j == 0), stop=False,
                                    perf_mode=DR)

                # fp8 ih matmul: one (hf, j) unit for packed gate gg
                def mm8(q, gidx, gg, hf, j, start, stop):
                    ps = gate_ps(q, gg)
                    col = gidx * 2048 + hf * 1024 + j * 256
                    lw = d["wih8"][q][:, col: col + 256]
                    rh = ln8[:, 2 * j * Bl:(2 * j + 2) * Bl]
                    nc.tensor.matmul(
                        ps[:, hf * Bl:(hf + 1) * Bl],
                        lw.rearrange("p (two m) -> p two m", two=2),
                        rh.rearrange("p (two n) -> p two n", two=2),
                        start=start, stop=stop, perf_mode=DR)

                # fp16 g-gate ih matmul: one (ic, hf) unit
                def mmg(q, ic, hf, start, stop):
                    ps = gate_ps(q, 1)
                    col = ic * 256 + hf * 128
                    nc.tensor.matmul(
                        ps[:, hf * Bl:(hf + 1) * Bl],
                        d["wihg"][q][:, col: col + P],
                        ln_t[:, ic * Bl:(ic + 1) * Bl],
                        start=start, stop=stop)

                # ---- hh for q0 (plus q1's i,g) queued before the stats
                # chain's consumers so the PE stays busy through it ------
                if use_hh:
                    emit_hh(0)
                    emit_hh(1, (0, 1))

                # ---- stats chain + matmul moving operands --------------
                # repeat: ln16/ln8 = 8*(mu-p)*rstd (sign folded into W)
                # first:  dds16/dd8 = (mu-p); rstd deferred past the psums
                # The in-order DVE queue is the critical path here: ops
                # that only need stat_ps (dds) must be queued BEFORE ops
                # that wait on the ACT chain (rawv/recip), or they stall.
                ln_t = sb.tile([P, BW], F16, name=f"ln{t}", tag="ln")
                ln8 = sb.tile([P, BW], F8, name=f"ln8_{t}", tag="ln8")
                musq2 = sb.tile([P, Bl], F32, name=f"mq{t}", tag="musq")
                rawv = sb.tile([P, Bl], F32, name=f"vr{t}", tag="vart")
                rstd_t = sb.tile([P, Bl], F16, name=f"rs{t}", tag="rstd")
                stdt = sb.tile([P, Bl], F32, name=f"sd{t}", tag="stdt")

                def emit_rstd_chain():
                    # raw = H*SS - S^2 = (H*std)^2
                    nc.scalar.square(musq2[:, :], stat_ps[:, 0:Bl])
                    nc.vector.scalar_tensor_tensor(
                        rawv[:, :], stat_ps[:, Bl:B2], float(H), musq2[:, :],
                        op0=OP.mult, op1=OP.subtract)
                    if use_hh:
                        # rstd_t = 8/std : ln16/ln8 carry 8x ln
                        nc.scalar.activation(stdt[:, :], rawv[:, :], AF.Square,
                                             bias=eps64_sb[:, 0:1],
                                             scale=float(1.0 / (64.0 * H * H)))
                    else:
                        # deferred: rstd_t = 1/(8*std) scales the gate psums
                        # (which carry 8x(p-mu)@Wgamma.T) after the matmuls
                        nc.scalar.activation(stdt[:, :], rawv[:, :], AF.Square,
                                             bias=epsx64_sb[:, 0:1],
                                             scale=float(64.0 / (H * H)))
                    with nc.allow_low_precision("f16 rstd: 5e-4 rel ok"):
                        nc.vector.reciprocal(rstd_t[:, :], stdt[:, :])

                if use_hh:
                    dds = []
                    for ic in range(2):
                        dd = sb.tile([P, Bl], F16, name=f"d{t}_{ic}", tag="lnd",
                                     bufs=3)
                        nc.vector.scalar_tensor_tensor(
                            dd[:, :], stat_ps[:, 0:Bl], 1.0 / H,
                            p16[:, ic, 0, :], op0=OP.mult, op1=OP.subtract)
                        dds.append(dd)
                    emit_rstd_chain()
                    for ic in range(NCH):
                        if ic >= 2:
                            dd = sb.tile([P, Bl], F16, name=f"d{t}_{ic}",
                                         tag="lnd", bufs=3)
                            nc.vector.scalar_tensor_tensor(
                                dd[:, :], stat_ps[:, 0:Bl], 1.0 / H,
                                p16[:, ic, 0, :], op0=OP.mult, op1=OP.subtract)
                        else:
                            dd = dds[ic]
                        # both on DVE: GPSIMD (which carries the v/xi
                        # updates) is ~2.5x slower per op and ln16 behind
                        # it would stall the q0 g-gate matmuls
                        nc.vector.tensor_mul(ln8[:, ic * Bl:(ic + 1) * Bl],
                                             dd[:, :], rstd_t[:, :])
                        nc.vector.tensor_mul(ln_t[:, ic * Bl:(ic + 1) * Bl],
                                             dd[:, :], rstd_t[:, :])
                else:
                    # dd8/dds16 straight off stat_ps (GPSIMD cannot read
                    # PSUM, so both on DVE); the rstd chain (only needed
                    # by the pointwise) queues after the first few
                    for ic in range(NCH):
                        nc.vector.scalar_tensor_tensor(
                            ln8[:, ic * Bl:(ic + 1) * Bl], stat_ps[:, 0:Bl],
                            1.0 / H, p16[:, ic, 0, :],
                            op0=OP.mult, op1=OP.subtract)
                        nc.vector.scalar_tensor_tensor(
                            ln_t[:, ic * Bl:(ic + 1) * Bl], stat_ps[:, 0:Bl],
                            1.0 / H, p16[:, ic, 0, :],
                            op0=OP.mult, op1=OP.subtract)
                        if ic == 3:
                            emit_rstd_chain()

                # ---- gates + LSTM pointwise, per quarter ---------------
                hnew = sb.tile([P, BW], F16, name=f"hn{t}", tag="hnew")
                h8new = (sb.tile([P, BW], F8, name=f"hn8_{t}", tag="h8new")
                         if st["store"] or st["pass_sbuf"] else None)
                f8g = F8GATES if use_hh else F8GATES[:2]   # first: i, o
                wa_t = d["wa"]
                pa_ps = {}

                def gact(dst, gg, func, q):
                    """activation from the gate psum (repeat steps)."""
                    gp = gates_q[q]
                    if biases_zero:
                        nc.scalar.activation(dst[:, :], gp[gg][:, :], func,
                                             scale=PS_INV)
                    else:
                        for hf in range(2):
                            hs = slice(hf * Bl, (hf + 1) * Bl)
                            nc.scalar.activation(dst[:, hs], gp[gg][:, hs],
                                                 func,
                                                 bias=gbias(s, gg, 2 * q + hf),
                                                 scale=PS_INV)

                def gact_first(dst, gg, func, q):
                    """deferred-rstd: DVE psum*rstd then activation."""
                    gp = gates_q[q]
                    gm = sb.tile([P, B2], F16, name=f"gm{t}_{q}_{gg}", tag="gm",
                                 bufs=3)
                    for hf in range(2):
                        hs = slice(hf * Bl, (hf + 1) * Bl)
                        nc.vector.tensor_mul(gm[:, hs], gp[gg][:, hs],
                                             rstd_t[:, :])
                    if biases_zero:
                        nc.scalar.activation(dst[:, :], gm[:, :], func)
                    else:
                        for hf in range(2):
                            hs = slice(hf * Bl, (hf + 1) * Bl)
                            nc.scalar.activation(dst[:, hs], gm[:, hs], func,
                                                 bias=gbias(s, gg, 2 * q + hf))

                for q in range(4):
                    if q == 0:
                        # j-outer: consume ln/dds pairs as the DVE produces
                        # them (fp8 pair j needs chunks 2j, 2j+1)
                        for j in range(4):
                            for gidx, gg in f8g:
                                for hf in range(2):
                                    mm8(0, gidx, gg, hf, j,
                                        start=(not use_hh and hf == 0 and j == 0),
                                        stop=(hf == 1 and j == 3))
                            for ic in (2 * j, 2 * j + 1):
                                for hf in range(2):
                                    mmg(0, ic, hf,
                                        start=(not use_hh and ic == 0 and hf == 0),
                                        stop=(ic == NCH - 1 and hf == 1))
                    else:
                        if use_hh:
                            emit_hh(q, (2, 3) if q == 1 else (0, 1, 2, 3))
                        # per-gate order (i, g, f, o): each gate's psum
                        # closes early so its activation (and the c/h
                        # chain) overlaps the remaining gates' matmuls
                        for gidx, gg in f8g[:1]:          # i
                            for hf in range(2):
                                for j in range(4):
                                    mm8(q, gidx, gg, hf, j,
                                        start=(not use_hh and hf == 0 and j == 0),
                                        stop=(hf == 1 and j == 3))
                        for ic in range(NCH):             # g (fp16)
                            for hf in range(2):
                                mmg(q, ic, hf,
                                    start=(not use_hh and ic == 0 and hf == 0),
                                    stop=(ic == NCH - 1 and hf == 1))
                        for gidx, gg in (f8g[2:] + f8g[1:2]):   # f then o
                            for hf in range(2):
                                for j in range(4):
                                    mm8(q, gidx, gg, hf, j,
                                        start=(not use_hh and hf == 0 and j == 0),
                                        stop=(hf == 1 and j == 3))
                    kick(2)
                    if q == 3:
                        # association pairs partial (K chunks 0..5, ready
                        # since quarter 2): covers the PE through the q3
                        # pointwise chain so wa doesn't stall on hnew.
                        # First steps get all 4 pairs (longer pointwise
                        # chain; only 3 gate psums live so PSUM fits);
                        # repeat steps 2 (4 gate psums live).
                        for pair in range(2 if use_hh else 4):
                            pa = pa_ps[pair] = psum.tile(
                                [P, B2], F32, name=f"pa{t}_{pair}", tag="mm")
                            for hf in range(2):
                                for ic in range(6):
                                    nc.tensor.matmul(
                                        pa[:, hf * Bl:(hf + 1) * Bl],
                                        wa_t[:, ic * H + (pair * 2 + hf) * P:
                                             ic * H + (pair * 2 + hf + 1) * P],
                                        hnew[:, ic * Bl:(ic + 1) * Bl],
                                        start=(hf == 0 and ic == 0),
                                        stop=False)
                    # pointwise for chunks hc = 2q, 2q+1. ACT queue order is
                    # (i, g, f, tanh(c), o) so the c chain pipelines behind
                    # the o-gate matmuls and h lands right after them.
                    qs = slice(2 * q * Bl, (2 * q + 2) * Bl)
                    tsi = sb.tile([P, B2], F16, name=f"tsi{t}_{q}", tag="tsi")
                    ttg = sb.tile([P, B2], F16, name=f"ttg{t}_{q}", tag="ttg")
                    tso = sb.tile([P, B2], F16, name=f"tso{t}_{q}", tag="tso")
                    ga = gact if use_hh else gact_first
                    ga(tsi, 0, AF.Sigmoid, q)
                    ga(ttg, 1, AF.Tanh, q)
                    if use_hh:
                        tsf = sb.tile([P, B2], F16, name=f"tsf{t}_{q}", tag="tsf")
                        ga(tsf, 3, AF.Sigmoid, q)
                        nc.vector.tensor_mul(tsi[:, :], tsi[:, :], ttg[:, :])
                        nc.vector.tensor_mul(tsf[:, :], tsf[:, :], c_t[:, qs])
                        nc.vector.tensor_add(c_t[:, qs], tsf[:, :], tsi[:, :])
                    else:
                        nc.vector.tensor_mul(c_t[:, qs], tsi[:, :], ttg[:, :])
                    ttc = sb.tile([P, B2], F16, name=f"ttc{t}_{q}", tag="ttc")
                    nc.scalar.activation(ttc[:, :], c_t[:, qs], AF.Tanh)
                    ga(tso, 2, AF.Sigmoid, q)
                    nc.vector.tensor_mul(hnew[:, qs], tso[:, :], ttc[:, :])
                    if h8new is not None:
                        nc.vector.scalar_tensor_tensor(
                            h8new[:, qs], tso[:, :], S_H, ttc[:, :],
                            op0=OP.mult, op1=OP.mult)

                sbuf_state[st["cell"]] = (hnew, h8new, c_t)

                # ---- LSTM state out -----------------------------------
                if st["store"]:
                    rs = st["rep_slot"]
                    nc.sync.dma_start(hst[rs][:, :], h8new[:, :])
                    nc.sync.dma_start(cst[rs][:, :], c_t[:, :])

                # ---- association: tanh(W_a @ h_new + b_a) --------------
                # ctx_t = 0.8^t * v_t ; v_t = v_{t-1} + 0.2*g*0.8^{-t}*tanh_t
                ccoef = float(0.2 * gate_sig[s] * (0.8 ** (-t)))
                acoef = float(0.2 * (0.8 ** t))
                if t + 1 < n_emit:
                    xi_t = sb.tile([P, BW], F16, name=f"xi{t + 1}", tag="xi")
                for pair in range(4):
                    if pair in pa_ps:
                        ps = pa_ps[pair]
                        ics = range(6, NCH)   # finish the partial pair
                    else:
                        ps = psum.tile([P, B2], F32, name=f"pa{t}_{pair}",
                                       tag="mm")
                        ics = range(NCH)
                    for hf in range(2):
                        oc = pair * 2 + hf
                        for ic in ics:
                            nc.tensor.matmul(
                                ps[:, hf * Bl:(hf + 1) * Bl],
                                wa_t[:, ic * H + oc * P: ic * H + (oc + 1) * P],
                                hnew[:, ic * Bl:(ic + 1) * Bl],
                                start=(pair not in pa_ps
                                       and hf == 0 and ic == 0),
                                stop=(hf == 1 and ic == NCH - 1))
                    # fused tail: tanh -> v update -> next xi
                    tnh = sb.tile([P, B2], F16, name=f"tnh{t}_{pair}", tag="tnh")
                    if biases_zero:
                        nc.scalar.activation(tnh[:, :], ps[:, :], AF.Tanh)
                    else:
                        for hf in range(2):
                            oc = pair * 2 + hf
                            nc.scalar.activation(tnh[:, hf * Bl:(hf + 1) * Bl],
                                                 ps[:, hf * Bl:(hf + 1) * Bl],
                                                 AF.Tanh, bias=bias1(ba_sb, s, oc))
                    cs = slice(pair * B2, (pair + 1) * B2)
                    if t == 0:
                        nc.vector.tensor_scalar_mul(v_sb[:, cs], tnh[:, :], ccoef)
                    else:
                        nc.vector.scalar_tensor_tensor(
                            v_sb[:, cs], tnh[:, :], ccoef, v_sb[:, cs],
                            op0=OP.mult, op1=OP.add)
                    if t + 1 < n_emit:
                        nc.vector.scalar_tensor_tensor(
                            xi_t[:, cs], v_sb[:, cs], acoef, x_sb[:, cs],
                            op0=OP.mult, op1=OP.add)
                    else:
                        # last step: stream the output per pair, overlapped
                        # with the remaining association work
                        nc.sync.dma_start(out_d[:, cs], v_sb[:, cs])
                    if pair == 3:
                        kick()    # wa(t+1)

                kick(8)   # flush any remaining prefetch jobs for t+1

    _split_multi_waits(nc)
    return nc


last_results = None   # BassKernelResults of the most recent run (for test.py)
last_nc = None
last_in_maps = None


def kernel(**inputs):
    n_exec = inputs.pop("_n_exec", None)
    n_reps = int(inputs.pop("_n_reps", 1))
    n_steps = int(inputs.pop("_n_steps", 0)) or None
    seq = [int(v) for v in np.asarray(inputs["cell_indices"]).reshape(-1)]
    if n_steps is None:
        n_steps = len(seq)
    seq = seq[:n_steps]

    x = np.asarray(inputs["x"], np.float32)
    B, Hd = x.shape
    assert Hd == H
    Bl = B // NCORES

    slots, rep_cells, plan = _analyze(seq)
    n_used, n_rep = len(slots), len(rep_cells)
    gl = np.asarray(inputs["gate_logit"], np.float64)
    gate_sig = [1.0 / (1.0 + np.exp(-gl[e])) for e in slots]

    packed = _host_pack(inputs, slots, rep_cells)
    nc = _build(plan, n_used, n_rep, Bl, gate_sig, n_steps,
                packed["biases_zero"], n_emit=n_exec, n_reps=n_reps)

    # per-core input maps (weights identical, x sliced)
    xT = np.ascontiguousarray(x.T)                       # [H, B]
    shared = dict(
        wp=packed["wp"], wa=packed["wa"], wihg=packed["wihg"],
        wih8=packed["wih8"],
        bp=packed["bp"], bg=packed["bg"], ba=packed["ba"])
    if n_rep:
        shared["whh"] = packed["whh"]
    in_maps = []
    for c in range(NCORES):
        xc = xT[:, c * Bl:(c + 1) * Bl]                  # [H, Bl]
        xc = np.ascontiguousarray(
            xc.reshape(NCH, P, Bl).transpose(1, 0, 2).reshape(P, NCH * Bl))
        m = dict(shared)
        m["xin"] = xc.astype(np.float16)
        in_maps.append(m)

    res = run_bass_kernel_spmd(nc, in_maps, core_ids=list(range(NCORES)),
                               trace=False)
    global last_results, last_nc, last_in_maps
    last_results = res
    last_nc = nc
    last_in_maps = in_maps

    scale = np.float64(0.8) ** (n_steps - 1)
    outs = []
    for c in range(NCORES):
        v = res.results[c]["out"]                        # [P, NCH*Bl]
        v = v.reshape(P, NCH, Bl).transpose(1, 0, 2).reshape(H, Bl)
        outs.append(v)
    full = np.concatenate(outs, axis=1)                  # [H, B]
    return np.ascontiguousarray((full.T.astype(np.float64) * scale).astype(np.float32))
